# revision 38
# baseline (speedup 1.0000x reference)
"""DAG-LSTM + MLP Trainium2 kernel.

Data-parallel over batch: 4096 rows -> 512 per NeuronCore x 8 cores, no
collectives.  The DAG structure (pred_idx / pred_mask) is read on the host at
call time and baked into the traced instruction stream: per-step predecessor
averaging becomes a short chain of scalar_tensor_tensor ops over only the
slots that are actually written and non-zero, and slot storage in SBUF is
allocated by liveness coloring.

Layouts (everything pre-transposed on the host so the device never
transposes):
  states h[s][l]  : SBUF [128, 2*512] fp16   col = hchunk*512 + batch
  states c[s][l]  : SBUF [128, 2*512] fp32
  weights W.T     : SBUF [128, nk*M]  fp16   col = kchunk*M + mcol
  gates (psum)    : PSUM [128, 512] per 128-wide 4H chunk, fp32
  gate activations: SBUF [128, 8*512] fp16   (i,i,f,f,g,g,o,o chunk order)

Engines: PE fp16 matmuls (1 cyc/row), ACT sigmoid/tanh with the (bih+bhh)
bias folded in, DVE h-path + gate combines (fp16 2x mode), GPSIMD c-path
predecessor aggregation (keeps DVE under the PE/ACT pace).
"""

import os
import sys

import numpy as np

for _p in ("/opt/trn_rl_repo",):
    if _p not in sys.path and os.path.isdir(_p):
        sys.path.insert(0, _p)

B, N, P = 4096, 24, 3
IN, H, L = 128, 256, 2
EXTRA, MH, OUT = 128, 512, 1
NCORES = 8
BC = B // NCORES            # 512 batch rows per core
G4 = 4 * H                  # 1024 gate width
NM = G4 // 128              # 8 gate chunks of 128
HC = H // 128               # 2 hidden chunks

_BUILD_CACHE = {}


def _chunk_k(wt: np.ndarray) -> np.ndarray:
    """[K, M] -> [128, (K//128)*M] with col = kchunk*M + m."""
    k, m = wt.shape
    nk = k // 128
    assert nk * 128 == k
    return np.ascontiguousarray(wt.reshape(nk, 128, m).transpose(1, 0, 2).reshape(128, nk * m))


def _dag_schedule(pred_idx: np.ndarray, pred_mask: np.ndarray):
    """Per step: (weights {slot: w/cnt for available non-zero slots}).

    Matches the reference exactly for arbitrary pred_idx/pred_mask: slot 0 and
    slots not yet written at step i read as zeros (dropped from the sum), but
    every mask unit still counts toward cnt.
    """
    sched = []
    for i in range(N):
        tot = 0.0
        w = {}
        for p in range(P):
            m = float(pred_mask[i, p])
            if m == 0.0:
                continue
            tot += m
            s = int(pred_idx[i, p])
            if 1 <= s <= i:                      # written and non-zero slot
                w[s] = w.get(s, 0.0) + m
        cnt = max(tot, 1.0)
        sched.append((w, cnt))
    return sched


def _needed_variants(sched):
    """cnt values > 1 that occur on steps with at least one live pred slot.
    For those steps the h predecessor sum is left UNSCALED and the matmul
    uses a host-prescaled Whh/cnt copy instead."""
    out = set()
    for w, cnt in sched:
        if w and cnt > 1.0:
            out.add(int(round(cnt)))
    return sorted(out)


def _color_slots(sched):
    """Greedy interval coloring of slots 1..N. Slot s is born at step s-1 and
    last read at max step using it (slot N also read by the final MLP)."""
    last = {}
    for i, (w, _cnt) in enumerate(sched):
        for s in w:
            last[s] = i
    last[N] = max(last.get(N, 0), N)             # final MLP reads h[N][1]
    color = {}
    free = []
    ncol = 0
    active = []                                   # (last_use, slot)
    for s in range(1, N + 1):
        born = s - 1
        still = []
        for lu, sl in active:
            if lu < born:
                free.append(color[sl])
            else:
                still.append((lu, sl))
        active = still
        if free:
            c = free.pop()
        else:
            c = ncol
            ncol += 1
        color[s] = c
        active.append((last.get(s, born - 1), s))
    return color, ncol, last


def _build(pred_idx: np.ndarray, pred_mask: np.ndarray):
    import concourse.bacc as bacc
    import concourse.tile as tile
    import concourse.mybir as mybir

    F16 = mybir.dt.float16
    F32 = mybir.dt.float32
    AF = mybir.ActivationFunctionType
    ALU = mybir.AluOpType

    sched = _dag_schedule(pred_idx, pred_mask)
    color, ncol, _last = _color_slots(sched)

    nc = bacc.Bacc("TRN2", target_bir_lowering=False, debug=False,
                   enable_asserts=False, num_devices=NCORES)

    # ---- DRAM parameters (per-core, preprocessed on host) -------------------
    d_dagsT = nc.dram_tensor("dagsT", [N, IN, BC], F16, kind="ExternalInput")
    d_featT = nc.dram_tensor("featT", [EXTRA, BC], F16, kind="ExternalInput")
    d_wihT0 = nc.dram_tensor("wihT0", [128, G4], F16, kind="ExternalInput")
    d_whhT0 = nc.dram_tensor("whhT0", [128, HC * G4], F16, kind="ExternalInput")
    d_wihT1 = nc.dram_tensor("wihT1", [128, HC * G4], F16, kind="ExternalInput")
    d_whhT1 = nc.dram_tensor("whhT1", [128, HC * G4], F16, kind="ExternalInput")
    variants = _needed_variants(sched)
    d_whh_v = {}
    for v in variants:
        for l in range(L):
            d_whh_v[(l, v)] = nc.dram_tensor(
                f"whhT{l}_v{v}", [128, HC * G4], F16, kind="ExternalInput")
    d_b0 = nc.dram_tensor("b0", [128, NM], F32, kind="ExternalInput")
    d_b1 = nc.dram_tensor("b1", [128, NM], F32, kind="ExternalInput")
    d_mw0T = nc.dram_tensor("mw0T", [128, 3 * MH], F16, kind="ExternalInput")
    d_mw1T = nc.dram_tensor("mw1T", [128, 4 * MH], F16, kind="ExternalInput")
    d_mw2T = nc.dram_tensor("mw2T", [128, 4], F16, kind="ExternalInput")
    d_mb0 = nc.dram_tensor("mb0", [128, 4], F32, kind="ExternalInput")
    d_mb1 = nc.dram_tensor("mb1", [128, 4], F32, kind="ExternalInput")
    d_mb2 = nc.dram_tensor("mb2", [128, 1], F32, kind="ExternalInput")
    d_out = nc.dram_tensor("out", [1, BC], F32, kind="ExternalOutput")

    with tile.TileContext(nc) as tc:
        from contextlib import ExitStack
        with ExitStack() as ctx:
            wpool = ctx.enter_context(tc.tile_pool(name="weights", bufs=1))
            spool = ctx.enter_context(tc.tile_pool(name="states", bufs=1))
            # high-color DAGs need the SBUF back for state tiles
            gpool = ctx.enter_context(
                tc.tile_pool(name="gact", bufs=4 if ncol <= 11 else 2))
            xpool = ctx.enter_context(tc.tile_pool(name="xin", bufs=3))
            kpool = ctx.enter_context(tc.tile_pool(name="work", bufs=3))
            apool = ctx.enter_context(tc.tile_pool(name="agg", bufs=4))
            ppool = ctx.enter_context(tc.tile_pool(name="psum", bufs=6, space="PSUM"))
            qpool = ctx.enter_context(tc.tile_pool(name="psum1", bufs=2, space="PSUM"))

            # ---- load weights ----------------------------------------------
            def wload(tag, dram, shape, dt):
                t = wpool.tile(shape, dt, tag=tag)
                nc.sync.dma_start(out=t[:, :], in_=dram[:, :])
                return t

            x_tiles = {}

            def fetch_x(i):
                if i < N and i not in x_tiles:
                    t = xpool.tile([128, BC], F16, tag="x")
                    nc.sync.dma_start(out=t[:, :], in_=d_dagsT[i])
                    x_tiles[i] = t

            # step-0 critical path first, then the rest
            wihT0 = wload("wihT0", d_wihT0, [128, G4], F16)
            b0 = wload("b0", d_b0, [128, NM], F32)
            fetch_x(0)
            whhT0 = wload("whhT0", d_whhT0, [128, HC * G4], F16)
            fetch_x(1)
            wihT1 = wload("wihT1", d_wihT1, [128, HC * G4], F16)
            whhT1 = wload("whhT1", d_whhT1, [128, HC * G4], F16)
            b1 = wload("b1", d_b1, [128, NM], F32)
            whh_v = {(0, 1): whhT0, (1, 1): whhT1}
            for (l, v), dram in d_whh_v.items():
                whh_v[(l, v)] = wload(f"whhT{l}_v{v}", dram,
                                      [128, HC * G4], F16)
            featT = wload("featT", d_featT, [EXTRA, BC], F16)
            mw0T = wload("mw0T", d_mw0T, [128, 3 * MH], F16)
            mw1T = wload("mw1T", d_mw1T, [128, 4 * MH], F16)
            mw2T = wload("mw2T", d_mw2T, [128, 4], F16)
            mb0 = wload("mb0", d_mb0, [128, 4], F32)
            mb1 = wload("mb1", d_mb1, [128, 4], F32)
            mb2 = wload("mb2", d_mb2, [128, 1], F32)

            h_tiles = {}                           # (slot, layer) -> tile
            c_tiles = {}
            xpre = {}                              # (step, chunk) -> open psum

            SIG = AF.Sigmoid
            TANH = AF.Tanh

            # PE warmup: ~5us of dummy matmuls during the initial weight DMA
            # wait so the HAM clock gate reaches 2.4 GHz before step 0.
            wu_src = kpool.tile([128, BC], F16, tag="wu")
            nc.vector.memset(wu_src[:, :], 0.0)
            for _ in range(3):
                wu_ps = ppool.tile([128, BC], F32, tag="gp0")
                for j in range(8):
                    nc.tensor.matmul(wu_ps[:, :], wu_src[:, 0:128],
                                     wu_src[:, :], start=(j == 0),
                                     stop=(j == 7))

            for i in range(N):
                fetch_x(i + 2)
                w, cnt = sched[i]
                slots = sorted(w.keys())
                # expand multiplicities m_s (integer mask weights)
                terms = []
                for s in slots:
                    terms += [s] * max(int(round(w[s])), 1)
                inv = 1.0 / cnt

                # predecessor aggregation for BOTH layers up-front (only needs
                # slots <= i, so it runs while earlier steps' matmuls stream).
                # GPSIMD only implements plain Add/Multiply, so build the
                # UNSCALED sum there; the 1/cnt scale folds into a DVE
                # tensor_scalar (h path) or the scalar_tensor_tensor that
                # computes sigf*c_in (c path).
                # steps whose preds include the slot written LAST step sit on
                # the critical path: aggregate those on DVE (3x faster per op
                # and no cross-engine hop after h2); the rest go to GPSIMD.
                hot = (i in slots)
                eng = nc.vector if hot else nc.gpsimd
                agg = []                   # per layer: (h_in UNSCALED, c_sum)
                for l in range(L):
                    h_in = None            # unscaled; matmul uses Whh/cnt
                    c_sum = None           # unscaled sum (or single)
                    if terms:
                        if len(terms) == 1:
                            h_in = h_tiles[(terms[0], l)]
                            c_sum = c_tiles[(terms[0], l)]
                        else:
                            acc_h = apool.tile([128, HC * BC], F16, tag="acch")
                            acc_c = apool.tile([128, HC * BC], F16, tag="accc")
                            if hot:
                                # half-granularity so the kc0 h-part matmuls
                                # start after half the chain
                                for kc in range(HC):
                                    sl = slice(kc * BC, (kc + 1) * BC)
                                    eng.tensor_add(
                                        acc_h[:, sl],
                                        h_tiles[(terms[0], l)][:, sl],
                                        h_tiles[(terms[1], l)][:, sl])
                                    for s in terms[2:]:
                                        eng.tensor_add(
                                            acc_h[:, sl], acc_h[:, sl],
                                            h_tiles[(s, l)][:, sl])
                                for kc in range(HC):
                                    sl = slice(kc * BC, (kc + 1) * BC)
                                    eng.tensor_add(
                                        acc_c[:, sl],
                                        c_tiles[(terms[0], l)][:, sl],
                                        c_tiles[(terms[1], l)][:, sl])
                                    for s in terms[2:]:
                                        eng.tensor_add(
                                            acc_c[:, sl], acc_c[:, sl],
                                            c_tiles[(s, l)][:, sl])
                            else:
                                eng.tensor_add(
                                    acc_h[:, :], h_tiles[(terms[0], l)][:, :],
                                    h_tiles[(terms[1], l)][:, :])
                                eng.tensor_add(
                                    acc_c[:, :], c_tiles[(terms[0], l)][:, :],
                                    c_tiles[(terms[1], l)][:, :])
                                for s in terms[2:]:
                                    eng.tensor_add(
                                        acc_h[:, :], acc_h[:, :],
                                        h_tiles[(s, l)][:, :])
                                    eng.tensor_add(
                                        acc_c[:, :], acc_c[:, :],
                                        c_tiles[(s, l)][:, :])
                            h_in = acc_h
                            c_sum = acc_c
                    agg.append((h_in, c_sum))

                h_l0_new = None
                vkey = int(round(cnt)) if (terms and cnt > 1.0) else 1
                for l in range(L):
                    wih = wihT0 if l == 0 else wihT1
                    whh = whh_v[(l, vkey)]
                    bias = b0 if l == 0 else b1
                    if l == 0:
                        x_chunks = [x_tiles[i][:, :]]
                    else:
                        x_chunks = [h_l0_new[:, kc * BC:(kc + 1) * BC]
                                    for kc in range(HC)]
                    h_in, c_sum = agg[l]

                    # gate matmuls + activations per 128-wide 4H chunk, in
                    # f,f,i,i,g,g,o,o order with the DVE combine interleaved
                    # so it starts while later chunks are still on the PE.
                    gact = gpool.tile([128, NM * BC], F16, tag="gact")

                    ptag = "gp0" if l == 0 else "gp1"

                    def emit_chunk(m):
                        pre = xpre.pop((i, m), None) if l == 0 else None
                        group = []
                        if pre is None:
                            pool = ppool if l == 0 else qpool
                            ps = pool.tile([128, BC], F32, tag=ptag)
                            for kc, xch in enumerate(x_chunks):
                                group.append((wih[:, kc * G4 + m * 128:
                                                  kc * G4 + (m + 1) * 128],
                                              xch, kc == 0))
                        else:
                            ps = pre            # x-part already accumulated
                        if h_in is not None:
                            for kc in range(HC):
                                group.append((whh[:, kc * G4 + m * 128:
                                                  kc * G4 + (m + 1) * 128],
                                              h_in[:, kc * BC:(kc + 1) * BC],
                                              False))
                        for j, (lhsT, rhs, st) in enumerate(group):
                            nc.tensor.matmul(ps[:, :], lhsT, rhs,
                                             start=st,
                                             stop=(j == len(group) - 1),
                                             skip_group_check=True)
                        func = TANH if m in (4, 5) else SIG
                        nc.scalar.activation(gact[:, m * BC:(m + 1) * BC],
                                             ps[:, :], func,
                                             bias=bias[:, m:m + 1])

                    sigi = gact[:, 0 * BC:2 * BC]
                    sigf = gact[:, 2 * BC:4 * BC]
                    tg = gact[:, 4 * BC:6 * BC]
                    sigo = gact[:, 6 * BC:8 * BC]
                    col = color[i + 1]
                    c_new = spool.tile([128, HC * BC], F16, tag=f"c{col}_{l}")

                    # The combine tail runs at H-chunk-half granularity: half
                    # kc only needs o-gate chunk 6+kc and produces the half of
                    # h_new that feeds the next layer's kc-chunk matmuls.
                    th = kpool.tile([128, HC * BC], F16, tag="th")
                    h_new = spool.tile([128, HC * BC], F16, tag=f"h{col}_{l}")
                    B2 = BC                        # 512 cols per half

                    def half(ap, kc):
                        return ap[:, kc * B2:(kc + 1) * B2]

                    if c_sum is None:
                        # no predecessors: c_in = 0, so sigf is irrelevant --
                        # skip the f-gate chunks (2,3) entirely.
                        for m in (0, 1, 4, 5):
                            emit_chunk(m)
                        for kc in range(HC):
                            nc.vector.tensor_mul(half(c_new, kc),
                                                 half(sigi, kc), half(tg, kc))
                            nc.scalar.activation(half(th, kc),
                                                 half(c_new, kc), TANH)
                            emit_chunk(6 + kc)
                            nc.vector.tensor_mul(half(h_new, kc),
                                                 half(sigo, kc), half(th, kc))
                    else:
                        # f gate FIRST so the c path starts while the rest of
                        # the gate chunks are still streaming on the PE.
                        for m in (2, 3):
                            emit_chunk(m)
                        for kc in range(HC):
                            sl = slice(kc * BC, (kc + 1) * BC)
                            if cnt == 1.0:
                                nc.vector.tensor_mul(c_new[:, sl],
                                                     sigf[:, sl],
                                                     c_sum[:, sl])
                            else:
                                # c_new = (c_sum * 1/cnt) * sigf, fused
                                nc.vector.scalar_tensor_tensor(
                                    c_new[:, sl], c_sum[:, sl], inv,
                                    sigf[:, sl], ALU.mult, ALU.mult)
                        for m in (0, 1, 4, 5):     # i and g gates
                            emit_chunk(m)
                        t2 = kpool.tile([128, HC * BC], F16, tag="t2")
                        for kc in range(HC):
                            nc.vector.tensor_mul(half(t2, kc),
                                                 half(sigi, kc), half(tg, kc))
                            nc.vector.tensor_add(half(c_new, kc),
                                                 half(c_new, kc),
                                                 half(t2, kc))
                            nc.scalar.activation(half(th, kc),
                                                 half(c_new, kc), TANH)
                            emit_chunk(6 + kc)
                            nc.vector.tensor_mul(half(h_new, kc),
                                                 half(sigo, kc), half(th, kc))

                    h_tiles[(i + 1, l)] = h_new
                    c_tiles[(i + 1, l)] = c_new
                    if l == 0:
                        h_l0_new = h_new
                        # Software-pipelined x-projection for step i+1 layer
                        # 0: depends only on the DMA'd x tile, so the PE can
                        # run it during this step's combine tails. The psum
                        # groups stay open; step i+1's h-part matmuls join
                        # them (start=False) and close the group.
                        if i + 1 < N:
                            w1 = sched[i + 1][0]
                            t1list = []
                            for s1 in sorted(w1):
                                t1list += [s1] * max(int(round(w1[s1])), 1)
                            pset = (2, 3, 0, 1) if t1list else (0, 1, 4, 5)
                            for m in pset:
                                ps = ppool.tile([128, BC], F32, tag="gp0")
                                nc.tensor.matmul(
                                    ps[:, :],
                                    wihT0[:, m * 128:(m + 1) * 128],
                                    x_tiles[i + 1][:, :], start=True,
                                    stop=(not t1list),
                                    skip_group_check=True)
                                xpre[(i + 1, m)] = ps

            # ---- MLP ------------------------------------------------------
            hlast = h_tiles[(N, L - 1)]
            fc_chunks = [hlast[:, 0:BC], hlast[:, BC:2 * BC], featT[:, :]]

            a0 = gpool.tile([128, 4 * BC], F16, tag="gact")
            for mo in range(4):
                ps = ppool.tile([128, BC], F32, tag="gp0")
                for j, fch in enumerate(fc_chunks):
                    nc.tensor.matmul(
                        ps[:, :],
                        mw0T[:, j * MH + mo * 128: j * MH + (mo + 1) * 128],
                        fch, start=(j == 0), stop=(j == len(fc_chunks) - 1))
                nc.scalar.activation(a0[:, mo * BC:(mo + 1) * BC], ps[:, :],
                                     AF.Relu, bias=mb0[:, mo:mo + 1])

            a1 = gpool.tile([128, 4 * BC], F16, tag="gact")
            for mo in range(4):
                ps = qpool.tile([128, BC], F32, tag="gp1")
                for kc in range(4):
                    nc.tensor.matmul(
                        ps[:, :],
                        mw1T[:, kc * MH + mo * 128: kc * MH + (mo + 1) * 128],
                        a0[:, kc * BC:(kc + 1) * BC],
                        start=(kc == 0), stop=(kc == 3))
                nc.scalar.activation(a1[:, mo * BC:(mo + 1) * BC], ps[:, :],
                                     AF.Relu, bias=mb1[:, mo:mo + 1])

            psf = ppool.tile([128, BC], F32, tag="gp0")
            for kc in range(4):
                nc.tensor.matmul(psf[:1, :], mw2T[:, kc:kc + 1],
                                 a1[:, kc * BC:(kc + 1) * BC],
                                 start=(kc == 0), stop=(kc == 3))
            out_sb = kpool.tile([128, BC], F32, tag="th")
            nc.scalar.activation(out_sb[:1, :], psf[:1, :], AF.Identity,
                                 bias=mb2[:1, 0:1])
            nc.sync.dma_start(out=d_out[:, :], in_=out_sb[:1, :])

    nc.compile()
    return nc


def _prep_core_inputs(inputs):
    """Host-side layout prep shared by all cores + per-core slices."""
    f16 = np.float16
    f32 = np.float32

    wihT0 = _chunk_k(np.ascontiguousarray(inputs["Wih0"].T)).astype(f16)
    whhT0f = _chunk_k(np.ascontiguousarray(inputs["Whh0"].T))
    whhT0 = whhT0f.astype(f16)
    wihT1 = _chunk_k(np.ascontiguousarray(inputs["Wih1"].T)).astype(f16)
    whhT1f = _chunk_k(np.ascontiguousarray(inputs["Whh1"].T))
    whhT1 = whhT1f.astype(f16)

    sched = _dag_schedule(np.asarray(inputs["pred_idx"], np.int32),
                          np.asarray(inputs["pred_mask"], np.int32))
    var_arrays = {}
    for v in _needed_variants(sched):
        var_arrays[f"whhT0_v{v}"] = (whhT0f / v).astype(f16)
        var_arrays[f"whhT1_v{v}"] = (whhT1f / v).astype(f16)
    b0 = np.ascontiguousarray((inputs["bih0"] + inputs["bhh0"])
                              .astype(f32).reshape(NM, 128).T)
    b1 = np.ascontiguousarray((inputs["bih1"] + inputs["bhh1"])
                              .astype(f32).reshape(NM, 128).T)
    mw0T = _chunk_k(np.ascontiguousarray(inputs["mW0"].T)).astype(f16)
    mw1T = _chunk_k(np.ascontiguousarray(inputs["mW1"].T)).astype(f16)
    mw2T = _chunk_k(np.ascontiguousarray(inputs["mW2"].T)).astype(f16)
    mb0 = np.ascontiguousarray(inputs["mb0"].astype(f32).reshape(4, 128).T)
    mb1 = np.ascontiguousarray(inputs["mb1"].astype(f32).reshape(4, 128).T)
    mb2 = np.zeros((128, 1), f32)
    mb2[0, 0] = np.float32(inputs["mb2"][0])

    shared = dict(wihT0=wihT0, whhT0=whhT0, wihT1=wihT1, whhT1=whhT1,
                  b0=b0, b1=b1, mw0T=mw0T, mw1T=mw1T, mw2T=mw2T,
                  mb0=mb0, mb1=mb1, mb2=mb2, **var_arrays)

    dags = np.asarray(inputs["dags"], np.float32)
    feats = np.asarray(inputs["features"], np.float32)
    in_maps = []
    for c in range(NCORES):
        lo, hi = c * BC, (c + 1) * BC
        dagsT = np.ascontiguousarray(
            dags[lo:hi].transpose(1, 2, 0)).astype(f16)      # [N, IN, BC]
        featT = np.ascontiguousarray(feats[lo:hi].T).astype(f16)
        m = dict(shared)
        m["dagsT"] = dagsT
        m["featT"] = featT
        in_maps.append(m)
    return in_maps


def _get_nc(pred_idx, pred_mask):
    key = (pred_idx.tobytes(), pred_mask.tobytes())
    if key not in _BUILD_CACHE:
        _BUILD_CACHE[key] = _build(pred_idx, pred_mask)
    return _BUILD_CACHE[key]


def run(inputs, trace=False):
    from concourse.bass_utils import run_bass_kernel_spmd

    pred_idx = np.asarray(inputs["pred_idx"], np.int32)
    pred_mask = np.asarray(inputs["pred_mask"], np.int32)
    nc = _get_nc(pred_idx, pred_mask)
    in_maps = _prep_core_inputs(inputs)
    res = run_bass_kernel_spmd(nc, in_maps, core_ids=list(range(NCORES)),
                               trace=trace)
    out = np.concatenate([np.asarray(r["out"], np.float32).reshape(BC)
                          for r in res.results])
    return out, res


def kernel(**inputs) -> np.ndarray:
    out, _ = run(inputs, trace=False)
    return out


# revision 43
# speedup vs baseline: 1.0042x; 1.0042x over previous
"""DAG-LSTM + MLP Trainium2 kernel.

Data-parallel over batch: 4096 rows -> 512 per NeuronCore x 8 cores, no
collectives.  The DAG structure (pred_idx / pred_mask) is read on the host at
call time and baked into the traced instruction stream: per-step predecessor
averaging becomes a short chain of scalar_tensor_tensor ops over only the
slots that are actually written and non-zero, and slot storage in SBUF is
allocated by liveness coloring.

Layouts (everything pre-transposed on the host so the device never
transposes):
  states h/c[s][l] : SBUF [128, 2*512] fp16   col = hchunk*512 + batch
  weights W.T      : SBUF [128, nk*M]  fp16   col = kchunk*M + mcol
  gates (psum)     : PSUM [128, 512] per 128-wide 4H chunk, fp32
  gate activations : SBUF [128, 8*512] fp16  (i,i,f,f,g,g,o,o chunk order)

Engines: PE fp16 matmuls (1 cyc/row, split psum rings per layer, software-
pipelined x-projections one step ahead, warmup spin for the HAM clock gate);
ACT sigmoid/tanh with the (bih+bhh) bias folded in, f-gate chunks first so
the c path starts early (f skipped entirely on no-predecessor steps); DVE
runs the combine tail at H-chunk-half granularity (fp16 2x mode); GPSIMD
does the unscaled predecessor sums (1/cnt lives in host-prescaled Whh/cnt
copies and the fused scalar_tensor_tensor), except on steps that read the
immediately-previous slot, which aggregate on DVE to shorten the critical
chain.
"""

import os
import sys

import numpy as np

for _p in ("/opt/trn_rl_repo",):
    if _p not in sys.path and os.path.isdir(_p):
        sys.path.insert(0, _p)

B, N, P = 4096, 24, 3
IN, H, L = 128, 256, 2
EXTRA, MH, OUT = 128, 512, 1
NCORES = 8
BC = B // NCORES            # 512 batch rows per core
G4 = 4 * H                  # 1024 gate width
NM = G4 // 128              # 8 gate chunks of 128
HC = H // 128               # 2 hidden chunks

_BUILD_CACHE = {}


def _chunk_k(wt: np.ndarray) -> np.ndarray:
    """[K, M] -> [128, (K//128)*M] with col = kchunk*M + m."""
    k, m = wt.shape
    nk = k // 128
    assert nk * 128 == k
    return np.ascontiguousarray(wt.reshape(nk, 128, m).transpose(1, 0, 2).reshape(128, nk * m))


def _dag_schedule(pred_idx: np.ndarray, pred_mask: np.ndarray):
    """Per step: (weights {slot: w/cnt for available non-zero slots}).

    Matches the reference exactly for arbitrary pred_idx/pred_mask: slot 0 and
    slots not yet written at step i read as zeros (dropped from the sum), but
    every mask unit still counts toward cnt.
    """
    sched = []
    for i in range(N):
        tot = 0.0
        w = {}
        for p in range(P):
            m = float(pred_mask[i, p])
            if m == 0.0:
                continue
            tot += m
            s = int(pred_idx[i, p])
            if 1 <= s <= i:                      # written and non-zero slot
                w[s] = w.get(s, 0.0) + m
        cnt = max(tot, 1.0)
        sched.append((w, cnt))
    return sched


def _needed_variants(sched):
    """cnt values > 1 that occur on steps with at least one live pred slot.
    For those steps the h predecessor sum is left UNSCALED and the matmul
    uses a host-prescaled Whh/cnt copy instead."""
    out = set()
    for w, cnt in sched:
        if w and cnt > 1.0:
            out.add(int(round(cnt)))
    return sorted(out)


def _color_slots(sched):
    """Greedy interval coloring of slots 1..N. Slot s is born at step s-1 and
    last read at max step using it (slot N also read by the final MLP)."""
    last = {}
    for i, (w, _cnt) in enumerate(sched):
        for s in w:
            last[s] = i
    last[N] = max(last.get(N, 0), N)             # final MLP reads h[N][1]
    color = {}
    free = []
    ncol = 0
    active = []                                   # (last_use, slot)
    for s in range(1, N + 1):
        born = s - 1
        still = []
        for lu, sl in active:
            if lu < born:
                free.append(color[sl])
            else:
                still.append((lu, sl))
        active = still
        if free:
            c = free.pop()
        else:
            c = ncol
            ncol += 1
        color[s] = c
        active.append((last.get(s, born - 1), s))
    return color, ncol, last


def _build(pred_idx: np.ndarray, pred_mask: np.ndarray):
    import concourse.bacc as bacc
    import concourse.tile as tile
    import concourse.mybir as mybir

    F16 = mybir.dt.float16
    F32 = mybir.dt.float32
    AF = mybir.ActivationFunctionType
    ALU = mybir.AluOpType

    sched = _dag_schedule(pred_idx, pred_mask)
    color, ncol, _last = _color_slots(sched)

    nc = bacc.Bacc("TRN2", target_bir_lowering=False, debug=False,
                   enable_asserts=False, num_devices=NCORES)

    # ---- DRAM parameters (per-core, preprocessed on host) -------------------
    d_dagsT = nc.dram_tensor("dagsT", [N, IN, BC], F16, kind="ExternalInput")
    d_featT = nc.dram_tensor("featT", [EXTRA, BC], F16, kind="ExternalInput")
    d_wihT0 = nc.dram_tensor("wihT0", [128, G4], F16, kind="ExternalInput")
    d_whhT0 = nc.dram_tensor("whhT0", [128, HC * G4], F16, kind="ExternalInput")
    d_wihT1 = nc.dram_tensor("wihT1", [128, HC * G4], F16, kind="ExternalInput")
    d_whhT1 = nc.dram_tensor("whhT1", [128, HC * G4], F16, kind="ExternalInput")
    variants = _needed_variants(sched)
    d_whh_v = {}
    for v in variants:
        for l in range(L):
            d_whh_v[(l, v)] = nc.dram_tensor(
                f"whhT{l}_v{v}", [128, HC * G4], F16, kind="ExternalInput")
    d_b0 = nc.dram_tensor("b0", [128, NM], F32, kind="ExternalInput")
    d_b1 = nc.dram_tensor("b1", [128, NM], F32, kind="ExternalInput")
    d_mw0T = nc.dram_tensor("mw0T", [128, 3 * MH], F16, kind="ExternalInput")
    d_mw1T = nc.dram_tensor("mw1T", [128, 4 * MH], F16, kind="ExternalInput")
    d_mw2T = nc.dram_tensor("mw2T", [128, 4], F16, kind="ExternalInput")
    d_mb0 = nc.dram_tensor("mb0", [128, 4], F32, kind="ExternalInput")
    d_mb1 = nc.dram_tensor("mb1", [128, 4], F32, kind="ExternalInput")
    d_mb2 = nc.dram_tensor("mb2", [128, 1], F32, kind="ExternalInput")
    d_out = nc.dram_tensor("out", [1, BC], F32, kind="ExternalOutput")

    with tile.TileContext(nc) as tc:
        from contextlib import ExitStack
        with ExitStack() as ctx:
            wpool = ctx.enter_context(tc.tile_pool(name="weights", bufs=1))
            spool = ctx.enter_context(tc.tile_pool(name="states", bufs=1))
            # high-color DAGs need the SBUF back for state tiles
            gpool = ctx.enter_context(
                tc.tile_pool(name="gact", bufs=4 if ncol <= 11 else 2))
            xpool = ctx.enter_context(tc.tile_pool(name="xin", bufs=3))
            kpool = ctx.enter_context(tc.tile_pool(name="work", bufs=3))
            apool = ctx.enter_context(tc.tile_pool(name="agg", bufs=4))
            ppool = ctx.enter_context(tc.tile_pool(name="psum", bufs=6, space="PSUM"))
            qpool = ctx.enter_context(tc.tile_pool(name="psum1", bufs=2, space="PSUM"))

            # ---- load weights ----------------------------------------------
            def wload(tag, dram, shape, dt):
                t = wpool.tile(shape, dt, tag=tag)
                nc.sync.dma_start(out=t[:, :], in_=dram[:, :])
                return t

            x_tiles = {}

            def fetch_x(i):
                if i < N and i not in x_tiles:
                    t = xpool.tile([128, BC], F16, tag="x")
                    nc.sync.dma_start(out=t[:, :], in_=d_dagsT[i])
                    x_tiles[i] = t

            # step-0 critical path first, then the rest
            wihT0 = wload("wihT0", d_wihT0, [128, G4], F16)
            b0 = wload("b0", d_b0, [128, NM], F32)
            fetch_x(0)
            whhT0 = wload("whhT0", d_whhT0, [128, HC * G4], F16)
            fetch_x(1)
            wihT1 = wload("wihT1", d_wihT1, [128, HC * G4], F16)
            whhT1 = wload("whhT1", d_whhT1, [128, HC * G4], F16)
            b1 = wload("b1", d_b1, [128, NM], F32)
            whh_v = {(0, 1): whhT0, (1, 1): whhT1}
            for (l, v), dram in d_whh_v.items():
                whh_v[(l, v)] = wload(f"whhT{l}_v{v}", dram,
                                      [128, HC * G4], F16)
            featT = wload("featT", d_featT, [EXTRA, BC], F16)
            mw0T = wload("mw0T", d_mw0T, [128, 3 * MH], F16)
            mw1T = wload("mw1T", d_mw1T, [128, 4 * MH], F16)
            mw2T = wload("mw2T", d_mw2T, [128, 4], F16)
            mb0 = wload("mb0", d_mb0, [128, 4], F32)
            mb1 = wload("mb1", d_mb1, [128, 4], F32)
            mb2 = wload("mb2", d_mb2, [128, 1], F32)

            h_tiles = {}                           # (slot, layer) -> tile
            c_tiles = {}
            xpre = {}                              # (step, chunk) -> open psum
            agg_tiles = {}                         # (step, layer) -> (h, c)

            SIG = AF.Sigmoid
            TANH = AF.Tanh

            def emit_agg(i2, l):
                """Predecessor aggregation for (step i2, layer l), emitted as
                soon as this layer's last needed slot exists. GPSIMD only
                implements plain Add/Multiply, so the sums are UNSCALED; the
                1/cnt scale lives in the prescaled Whh/cnt weights (h path)
                and the fused scalar_tensor_tensor (c path). Steps that read
                the slot written one step earlier are on the critical path:
                those aggregate on DVE, half-split, right behind h2."""
                w2, _c2 = sched[i2]
                slots2 = sorted(w2.keys())
                terms2 = []
                for s in slots2:
                    terms2 += [s] * max(int(round(w2[s])), 1)
                h_in = None
                c_sum = None
                if terms2:
                    if len(terms2) == 1:
                        h_in = h_tiles[(terms2[0], l)]
                        c_sum = c_tiles[(terms2[0], l)]
                    else:
                        hot2 = (i2 in slots2)
                        eng = nc.vector if hot2 else nc.gpsimd
                        acc_h = apool.tile([128, HC * BC], F16, tag="acch")
                        acc_c = apool.tile([128, HC * BC], F16, tag="accc")
                        if hot2:
                            for kc in range(HC):
                                sl = slice(kc * BC, (kc + 1) * BC)
                                eng.tensor_add(
                                    acc_h[:, sl],
                                    h_tiles[(terms2[0], l)][:, sl],
                                    h_tiles[(terms2[1], l)][:, sl])
                                for s in terms2[2:]:
                                    eng.tensor_add(
                                        acc_h[:, sl], acc_h[:, sl],
                                        h_tiles[(s, l)][:, sl])
                            for kc in range(HC):
                                sl = slice(kc * BC, (kc + 1) * BC)
                                eng.tensor_add(
                                    acc_c[:, sl],
                                    c_tiles[(terms2[0], l)][:, sl],
                                    c_tiles[(terms2[1], l)][:, sl])
                                for s in terms2[2:]:
                                    eng.tensor_add(
                                        acc_c[:, sl], acc_c[:, sl],
                                        c_tiles[(s, l)][:, sl])
                        else:
                            eng.tensor_add(
                                acc_h[:, :], h_tiles[(terms2[0], l)][:, :],
                                h_tiles[(terms2[1], l)][:, :])
                            eng.tensor_add(
                                acc_c[:, :], c_tiles[(terms2[0], l)][:, :],
                                c_tiles[(terms2[1], l)][:, :])
                            for s in terms2[2:]:
                                eng.tensor_add(
                                    acc_h[:, :], acc_h[:, :],
                                    h_tiles[(s, l)][:, :])
                                eng.tensor_add(
                                    acc_c[:, :], acc_c[:, :],
                                    c_tiles[(s, l)][:, :])
                        h_in = acc_h
                        c_sum = acc_c
                agg_tiles[(i2, l)] = (h_in, c_sum)

            # PE warmup: ~5us of dummy matmuls during the initial weight DMA
            # wait so the HAM clock gate reaches 2.4 GHz before step 0.
            wu_src = kpool.tile([128, BC], F16, tag="wu")
            nc.vector.memset(wu_src[:, :], 0.0)
            for _ in range(3):
                wu_ps = ppool.tile([128, BC], F32, tag="gp0")
                for j in range(8):
                    nc.tensor.matmul(wu_ps[:, :], wu_src[:, 0:128],
                                     wu_src[:, :], start=(j == 0),
                                     stop=(j == 7))

            for i in range(N):
                fetch_x(i + 2)
                w, cnt = sched[i]
                slots = sorted(w.keys())
                # expand multiplicities m_s (integer mask weights)
                terms = []
                for s in slots:
                    terms += [s] * max(int(round(w[s])), 1)
                inv = 1.0 / cnt

                # (the aggregation for this step was emitted at the end of
                # the PREVIOUS step's matching layer body via emit_agg, so
                # its adds queue right behind the producing h2/c2 and run
                # one layer earlier)
                if i == 0:
                    emit_agg(0, 0)
                    emit_agg(0, 1)

                h_l0_new = None
                vkey = int(round(cnt)) if (terms and cnt > 1.0) else 1
                for l in range(L):
                    wih = wihT0 if l == 0 else wihT1
                    whh = whh_v[(l, vkey)]
                    bias = b0 if l == 0 else b1
                    if l == 0:
                        x_chunks = [x_tiles[i][:, :]]
                    else:
                        x_chunks = [h_l0_new[:, kc * BC:(kc + 1) * BC]
                                    for kc in range(HC)]
                    h_in, c_sum = agg_tiles.pop((i, l))

                    # gate matmuls + activations per 128-wide 4H chunk, in
                    # f,f,i,i,g,g,o,o order with the DVE combine interleaved
                    # so it starts while later chunks are still on the PE.
                    gact = gpool.tile([128, NM * BC], F16, tag="gact")

                    ptag = "gp0" if l == 0 else "gp1"

                    def emit_chunk(m):
                        pre = xpre.pop((i, m), None) if l == 0 else None
                        group = []
                        if pre is None:
                            pool = ppool if l == 0 else qpool
                            ps = pool.tile([128, BC], F32, tag=ptag)
                            for kc, xch in enumerate(x_chunks):
                                group.append((wih[:, kc * G4 + m * 128:
                                                  kc * G4 + (m + 1) * 128],
                                              xch, kc == 0))
                        else:
                            ps = pre            # x-part already accumulated
                        if h_in is not None:
                            for kc in range(HC):
                                group.append((whh[:, kc * G4 + m * 128:
                                                  kc * G4 + (m + 1) * 128],
                                              h_in[:, kc * BC:(kc + 1) * BC],
                                              False))
                        for j, (lhsT, rhs, st) in enumerate(group):
                            nc.tensor.matmul(ps[:, :], lhsT, rhs,
                                             start=st,
                                             stop=(j == len(group) - 1),
                                             skip_group_check=True)
                        func = TANH if m in (4, 5) else SIG
                        nc.scalar.activation(gact[:, m * BC:(m + 1) * BC],
                                             ps[:, :], func,
                                             bias=bias[:, m:m + 1])

                    sigi = gact[:, 0 * BC:2 * BC]
                    sigf = gact[:, 2 * BC:4 * BC]
                    tg = gact[:, 4 * BC:6 * BC]
                    sigo = gact[:, 6 * BC:8 * BC]
                    col = color[i + 1]
                    c_new = spool.tile([128, HC * BC], F16, tag=f"c{col}_{l}")

                    # The combine tail runs at H-chunk-half granularity: half
                    # kc only needs o-gate chunk 6+kc and produces the half of
                    # h_new that feeds the next layer's kc-chunk matmuls.
                    th = kpool.tile([128, HC * BC], F16, tag="th")
                    h_new = spool.tile([128, HC * BC], F16, tag=f"h{col}_{l}")
                    B2 = BC                        # 512 cols per half

                    def half(ap, kc):
                        return ap[:, kc * B2:(kc + 1) * B2]

                    if c_sum is None:
                        # no predecessors: c_in = 0, so sigf is irrelevant --
                        # skip the f-gate chunks (2,3) entirely.
                        for m in (0, 1, 4, 5):
                            emit_chunk(m)
                        for kc in range(HC):
                            nc.vector.tensor_mul(half(c_new, kc),
                                                 half(sigi, kc), half(tg, kc))
                            nc.scalar.activation(half(th, kc),
                                                 half(c_new, kc), TANH)
                            emit_chunk(6 + kc)
                            nc.vector.tensor_mul(half(h_new, kc),
                                                 half(sigo, kc), half(th, kc))
                    else:
                        # f gate FIRST so the c path starts while the rest of
                        # the gate chunks are still streaming on the PE.
                        for m in (2, 3):
                            emit_chunk(m)
                        for kc in range(HC):
                            sl = slice(kc * BC, (kc + 1) * BC)
                            if cnt == 1.0:
                                nc.vector.tensor_mul(c_new[:, sl],
                                                     sigf[:, sl],
                                                     c_sum[:, sl])
                            else:
                                # c_new = (c_sum * 1/cnt) * sigf, fused
                                nc.vector.scalar_tensor_tensor(
                                    c_new[:, sl], c_sum[:, sl], inv,
                                    sigf[:, sl], ALU.mult, ALU.mult)
                        for m in (0, 1, 4, 5):     # i and g gates
                            emit_chunk(m)
                        t2 = kpool.tile([128, HC * BC], F16, tag="t2")
                        for kc in range(HC):
                            nc.vector.tensor_mul(half(t2, kc),
                                                 half(sigi, kc), half(tg, kc))
                            nc.vector.tensor_add(half(c_new, kc),
                                                 half(c_new, kc),
                                                 half(t2, kc))
                            nc.scalar.activation(half(th, kc),
                                                 half(c_new, kc), TANH)
                            emit_chunk(6 + kc)
                            nc.vector.tensor_mul(half(h_new, kc),
                                                 half(sigo, kc), half(th, kc))

                    h_tiles[(i + 1, l)] = h_new
                    c_tiles[(i + 1, l)] = c_new
                    if i + 1 < N:
                        emit_agg(i + 1, l)
                    if l == 0:
                        h_l0_new = h_new
                        # Software-pipelined x-projection for step i+1 layer
                        # 0: depends only on the DMA'd x tile, so the PE can
                        # run it during this step's combine tails. The psum
                        # groups stay open; step i+1's h-part matmuls join
                        # them (start=False) and close the group.
                        if i + 1 < N:
                            w1 = sched[i + 1][0]
                            t1list = []
                            for s1 in sorted(w1):
                                t1list += [s1] * max(int(round(w1[s1])), 1)
                            pset = (2, 3, 0, 1) if t1list else (0, 1, 4, 5)
                            for m in pset:
                                ps = ppool.tile([128, BC], F32, tag="gp0")
                                nc.tensor.matmul(
                                    ps[:, :],
                                    wihT0[:, m * 128:(m + 1) * 128],
                                    x_tiles[i + 1][:, :], start=True,
                                    stop=(not t1list),
                                    skip_group_check=True)
                                xpre[(i + 1, m)] = ps

            # ---- MLP ------------------------------------------------------
            hlast = h_tiles[(N, L - 1)]
            fc_chunks = [hlast[:, 0:BC], hlast[:, BC:2 * BC], featT[:, :]]

            a0 = gpool.tile([128, 4 * BC], F16, tag="gact")
            for mo in range(4):
                ps = ppool.tile([128, BC], F32, tag="gp0")
                for j, fch in enumerate(fc_chunks):
                    nc.tensor.matmul(
                        ps[:, :],
                        mw0T[:, j * MH + mo * 128: j * MH + (mo + 1) * 128],
                        fch, start=(j == 0), stop=(j == len(fc_chunks) - 1))
                nc.scalar.activation(a0[:, mo * BC:(mo + 1) * BC], ps[:, :],
                                     AF.Relu, bias=mb0[:, mo:mo + 1])

            a1 = gpool.tile([128, 4 * BC], F16, tag="gact")
            for mo in range(4):
                ps = qpool.tile([128, BC], F32, tag="gp1")
                for kc in range(4):
                    nc.tensor.matmul(
                        ps[:, :],
                        mw1T[:, kc * MH + mo * 128: kc * MH + (mo + 1) * 128],
                        a0[:, kc * BC:(kc + 1) * BC],
                        start=(kc == 0), stop=(kc == 3))
                nc.scalar.activation(a1[:, mo * BC:(mo + 1) * BC], ps[:, :],
                                     AF.Relu, bias=mb1[:, mo:mo + 1])

            psf = ppool.tile([128, BC], F32, tag="gp0")
            for kc in range(4):
                nc.tensor.matmul(psf[:1, :], mw2T[:, kc:kc + 1],
                                 a1[:, kc * BC:(kc + 1) * BC],
                                 start=(kc == 0), stop=(kc == 3))
            out_sb = kpool.tile([128, BC], F32, tag="th")
            nc.scalar.activation(out_sb[:1, :], psf[:1, :], AF.Identity,
                                 bias=mb2[:1, 0:1])
            nc.sync.dma_start(out=d_out[:, :], in_=out_sb[:1, :])

    nc.compile()
    return nc


def _prep_core_inputs(inputs):
    """Host-side layout prep shared by all cores + per-core slices."""
    f16 = np.float16
    f32 = np.float32

    wihT0 = _chunk_k(np.ascontiguousarray(inputs["Wih0"].T)).astype(f16)
    whhT0f = _chunk_k(np.ascontiguousarray(inputs["Whh0"].T))
    whhT0 = whhT0f.astype(f16)
    wihT1 = _chunk_k(np.ascontiguousarray(inputs["Wih1"].T)).astype(f16)
    whhT1f = _chunk_k(np.ascontiguousarray(inputs["Whh1"].T))
    whhT1 = whhT1f.astype(f16)

    sched = _dag_schedule(np.asarray(inputs["pred_idx"], np.int32),
                          np.asarray(inputs["pred_mask"], np.int32))
    var_arrays = {}
    for v in _needed_variants(sched):
        var_arrays[f"whhT0_v{v}"] = (whhT0f / v).astype(f16)
        var_arrays[f"whhT1_v{v}"] = (whhT1f / v).astype(f16)
    b0 = np.ascontiguousarray((inputs["bih0"] + inputs["bhh0"])
                              .astype(f32).reshape(NM, 128).T)
    b1 = np.ascontiguousarray((inputs["bih1"] + inputs["bhh1"])
                              .astype(f32).reshape(NM, 128).T)
    mw0T = _chunk_k(np.ascontiguousarray(inputs["mW0"].T)).astype(f16)
    mw1T = _chunk_k(np.ascontiguousarray(inputs["mW1"].T)).astype(f16)
    mw2T = _chunk_k(np.ascontiguousarray(inputs["mW2"].T)).astype(f16)
    mb0 = np.ascontiguousarray(inputs["mb0"].astype(f32).reshape(4, 128).T)
    mb1 = np.ascontiguousarray(inputs["mb1"].astype(f32).reshape(4, 128).T)
    mb2 = np.zeros((128, 1), f32)
    mb2[0, 0] = np.float32(inputs["mb2"][0])

    shared = dict(wihT0=wihT0, whhT0=whhT0, wihT1=wihT1, whhT1=whhT1,
                  b0=b0, b1=b1, mw0T=mw0T, mw1T=mw1T, mw2T=mw2T,
                  mb0=mb0, mb1=mb1, mb2=mb2, **var_arrays)

    dags = np.asarray(inputs["dags"], np.float32)
    feats = np.asarray(inputs["features"], np.float32)
    in_maps = []
    for c in range(NCORES):
        lo, hi = c * BC, (c + 1) * BC
        dagsT = np.ascontiguousarray(
            dags[lo:hi].transpose(1, 2, 0)).astype(f16)      # [N, IN, BC]
        featT = np.ascontiguousarray(feats[lo:hi].T).astype(f16)
        m = dict(shared)
        m["dagsT"] = dagsT
        m["featT"] = featT
        in_maps.append(m)
    return in_maps


def _get_nc(pred_idx, pred_mask):
    key = (pred_idx.tobytes(), pred_mask.tobytes())
    if key not in _BUILD_CACHE:
        _BUILD_CACHE[key] = _build(pred_idx, pred_mask)
    return _BUILD_CACHE[key]


def run(inputs, trace=False):
    from concourse.bass_utils import run_bass_kernel_spmd

    pred_idx = np.asarray(inputs["pred_idx"], np.int32)
    pred_mask = np.asarray(inputs["pred_mask"], np.int32)
    nc = _get_nc(pred_idx, pred_mask)
    in_maps = _prep_core_inputs(inputs)
    res = run_bass_kernel_spmd(nc, in_maps, core_ids=list(range(NCORES)),
                               trace=trace)
    out = np.concatenate([np.asarray(r["out"], np.float32).reshape(BC)
                          for r in res.results])
    return out, res


def kernel(**inputs) -> np.ndarray:
    out, _ = run(inputs, trace=False)
    return out


# revision 45
# speedup vs baseline: 2.2397x; 2.2303x over previous
"""DAG-LSTM + MLP Trainium2 kernel.

Data-parallel over batch: 4096 rows -> 512 per NeuronCore x 8 cores, no
collectives.  The DAG structure (pred_idx / pred_mask) is read on the host at
call time and baked into the traced instruction stream: per-step predecessor
averaging becomes a short chain of scalar_tensor_tensor ops over only the
slots that are actually written and non-zero, and slot storage in SBUF is
allocated by liveness coloring.

Layouts (everything pre-transposed on the host so the device never
transposes):
  states h/c[s][l] : SBUF [128, 2*512] fp16   col = hchunk*512 + batch
  weights W.T      : SBUF [128, nk*M]  fp16   col = kchunk*M + mcol
  gates (psum)     : PSUM [128, 512] per 128-wide 4H chunk, fp32
  gate activations : SBUF [128, 8*512] fp16  (i,i,f,f,g,g,o,o chunk order)

Engines: PE fp16 matmuls (1 cyc/row, split psum rings per layer, software-
pipelined x-projections one step ahead, warmup spin for the HAM clock gate);
ACT sigmoid/tanh with the (bih+bhh) bias folded in, f-gate chunks first so
the c path starts early (f skipped entirely on no-predecessor steps); DVE
runs the combine tail at H-chunk-half granularity (fp16 2x mode); GPSIMD
does the unscaled predecessor sums (1/cnt lives in host-prescaled Whh/cnt
copies and the fused scalar_tensor_tensor), except on steps that read the
immediately-previous slot, which aggregate on DVE to shorten the critical
chain.
"""

import os
import sys

import numpy as np

for _p in ("/opt/trn_rl_repo",):
    if _p not in sys.path and os.path.isdir(_p):
        sys.path.insert(0, _p)

B, N, P = 4096, 24, 3
IN, H, L = 128, 256, 2
EXTRA, MH, OUT = 128, 512, 1
NCORES = 8
BC = B // NCORES            # 512 batch rows per core
G4 = 4 * H                  # 1024 gate width
NM = G4 // 128              # 8 gate chunks of 128
HC = H // 128               # 2 hidden chunks

_BUILD_CACHE = {}


def _chunk_k(wt: np.ndarray) -> np.ndarray:
    """[K, M] -> [128, (K//128)*M] with col = kchunk*M + m."""
    k, m = wt.shape
    nk = k // 128
    assert nk * 128 == k
    return np.ascontiguousarray(wt.reshape(nk, 128, m).transpose(1, 0, 2).reshape(128, nk * m))


def _dag_schedule(pred_idx: np.ndarray, pred_mask: np.ndarray):
    """Per step: (weights {slot: w/cnt for available non-zero slots}).

    Matches the reference exactly for arbitrary pred_idx/pred_mask: slot 0 and
    slots not yet written at step i read as zeros (dropped from the sum), but
    every mask unit still counts toward cnt.
    """
    sched = []
    for i in range(N):
        tot = 0.0
        w = {}
        for p in range(P):
            m = float(pred_mask[i, p])
            if m == 0.0:
                continue
            tot += m
            s = int(pred_idx[i, p])
            if 1 <= s <= i:                      # written and non-zero slot
                w[s] = w.get(s, 0.0) + m
        cnt = max(tot, 1.0)
        sched.append((w, cnt))
    return sched


def _needed_variants(sched):
    """cnt values > 1 that occur on steps with at least one live pred slot.
    For those steps the h predecessor sum is left UNSCALED and the matmul
    uses a host-prescaled Whh/cnt copy instead."""
    out = set()
    for w, cnt in sched:
        if w and cnt > 1.0:
            out.add(int(round(cnt)))
    return sorted(out)


def _live_steps(sched):
    """Backward reachability from the output node: only steps whose slot is
    (transitively) read on the path to slot N (the MLP input) need to run.
    Skipping the rest is exact -- their values are unobservable."""
    live = set()
    stack = [N - 1]
    while stack:
        i = stack.pop()
        if i in live:
            continue
        live.add(i)
        for s in sched[i][0]:          # live pred slots; producer = step s-1
            if s - 1 not in live:
                stack.append(s - 1)
    return live


def _color_slots(sched):
    """Greedy interval coloring of slots 1..N. Slot s is born at step s-1 and
    last read at max step using it (slot N also read by the final MLP)."""
    last = {}
    for i, (w, _cnt) in enumerate(sched):
        for s in w:
            last[s] = i
    last[N] = max(last.get(N, 0), N)             # final MLP reads h[N][1]
    color = {}
    free = []
    ncol = 0
    active = []                                   # (last_use, slot)
    for s in range(1, N + 1):
        born = s - 1
        still = []
        for lu, sl in active:
            if lu < born:
                free.append(color[sl])
            else:
                still.append((lu, sl))
        active = still
        if free:
            c = free.pop()
        else:
            c = ncol
            ncol += 1
        color[s] = c
        active.append((last.get(s, born - 1), s))
    return color, ncol, last


def _build(pred_idx: np.ndarray, pred_mask: np.ndarray):
    import concourse.bacc as bacc
    import concourse.tile as tile
    import concourse.mybir as mybir

    F16 = mybir.dt.float16
    F32 = mybir.dt.float32
    AF = mybir.ActivationFunctionType
    ALU = mybir.AluOpType

    sched = _dag_schedule(pred_idx, pred_mask)
    color, ncol, _last = _color_slots(sched)
    live_list = sorted(_live_steps(sched))

    nc = bacc.Bacc("TRN2", target_bir_lowering=False, debug=False,
                   enable_asserts=False, num_devices=NCORES)

    # ---- DRAM parameters (per-core, preprocessed on host) -------------------
    d_dagsT = nc.dram_tensor("dagsT", [N, IN, BC], F16, kind="ExternalInput")
    d_featT = nc.dram_tensor("featT", [EXTRA, BC], F16, kind="ExternalInput")
    d_wihT0 = nc.dram_tensor("wihT0", [128, G4], F16, kind="ExternalInput")
    d_whhT0 = nc.dram_tensor("whhT0", [128, HC * G4], F16, kind="ExternalInput")
    d_wihT1 = nc.dram_tensor("wihT1", [128, HC * G4], F16, kind="ExternalInput")
    d_whhT1 = nc.dram_tensor("whhT1", [128, HC * G4], F16, kind="ExternalInput")
    variants = _needed_variants(sched)
    d_whh_v = {}
    for v in variants:
        for l in range(L):
            d_whh_v[(l, v)] = nc.dram_tensor(
                f"whhT{l}_v{v}", [128, HC * G4], F16, kind="ExternalInput")
    d_b0 = nc.dram_tensor("b0", [128, NM], F32, kind="ExternalInput")
    d_b1 = nc.dram_tensor("b1", [128, NM], F32, kind="ExternalInput")
    d_mw0T = nc.dram_tensor("mw0T", [128, 3 * MH], F16, kind="ExternalInput")
    d_mw1T = nc.dram_tensor("mw1T", [128, 4 * MH], F16, kind="ExternalInput")
    d_mw2T = nc.dram_tensor("mw2T", [128, 4], F16, kind="ExternalInput")
    d_mb0 = nc.dram_tensor("mb0", [128, 4], F32, kind="ExternalInput")
    d_mb1 = nc.dram_tensor("mb1", [128, 4], F32, kind="ExternalInput")
    d_mb2 = nc.dram_tensor("mb2", [128, 1], F32, kind="ExternalInput")
    d_out = nc.dram_tensor("out", [1, BC], F32, kind="ExternalOutput")

    with tile.TileContext(nc) as tc:
        from contextlib import ExitStack
        with ExitStack() as ctx:
            wpool = ctx.enter_context(tc.tile_pool(name="weights", bufs=1))
            spool = ctx.enter_context(tc.tile_pool(name="states", bufs=1))
            # high-color DAGs need the SBUF back for state tiles
            gpool = ctx.enter_context(
                tc.tile_pool(name="gact", bufs=4 if ncol <= 11 else 2))
            xpool = ctx.enter_context(tc.tile_pool(name="xin", bufs=3))
            kpool = ctx.enter_context(tc.tile_pool(name="work", bufs=3))
            apool = ctx.enter_context(tc.tile_pool(name="agg", bufs=4))
            ppool = ctx.enter_context(tc.tile_pool(name="psum", bufs=6, space="PSUM"))
            qpool = ctx.enter_context(tc.tile_pool(name="psum1", bufs=2, space="PSUM"))

            # ---- load weights ----------------------------------------------
            def wload(tag, dram, shape, dt):
                t = wpool.tile(shape, dt, tag=tag)
                nc.sync.dma_start(out=t[:, :], in_=dram[:, :])
                return t

            x_tiles = {}

            def fetch_x(i):
                if i < N and i not in x_tiles:
                    t = xpool.tile([128, BC], F16, tag="x")
                    nc.sync.dma_start(out=t[:, :], in_=d_dagsT[i])
                    x_tiles[i] = t

            # step-0 critical path first, then the rest
            wihT0 = wload("wihT0", d_wihT0, [128, G4], F16)
            b0 = wload("b0", d_b0, [128, NM], F32)
            fetch_x(live_list[0])
            whhT0 = wload("whhT0", d_whhT0, [128, HC * G4], F16)
            if len(live_list) > 1:
                fetch_x(live_list[1])
            wihT1 = wload("wihT1", d_wihT1, [128, HC * G4], F16)
            whhT1 = wload("whhT1", d_whhT1, [128, HC * G4], F16)
            b1 = wload("b1", d_b1, [128, NM], F32)
            whh_v = {(0, 1): whhT0, (1, 1): whhT1}
            for (l, v), dram in d_whh_v.items():
                whh_v[(l, v)] = wload(f"whhT{l}_v{v}", dram,
                                      [128, HC * G4], F16)
            featT = wload("featT", d_featT, [EXTRA, BC], F16)
            mw0T = wload("mw0T", d_mw0T, [128, 3 * MH], F16)
            mw1T = wload("mw1T", d_mw1T, [128, 4 * MH], F16)
            mw2T = wload("mw2T", d_mw2T, [128, 4], F16)
            mb0 = wload("mb0", d_mb0, [128, 4], F32)
            mb1 = wload("mb1", d_mb1, [128, 4], F32)
            mb2 = wload("mb2", d_mb2, [128, 1], F32)

            h_tiles = {}                           # (slot, layer) -> tile
            c_tiles = {}
            xpre = {}                              # (step, chunk) -> open psum
            agg_tiles = {}                         # (step, layer) -> (h, c)

            SIG = AF.Sigmoid
            TANH = AF.Tanh

            def emit_agg(i2, l):
                """Predecessor aggregation for (step i2, layer l), emitted as
                soon as this layer's last needed slot exists. GPSIMD only
                implements plain Add/Multiply, so the sums are UNSCALED; the
                1/cnt scale lives in the prescaled Whh/cnt weights (h path)
                and the fused scalar_tensor_tensor (c path). Steps that read
                the slot written one step earlier are on the critical path:
                those aggregate on DVE, half-split, right behind h2."""
                w2, _c2 = sched[i2]
                slots2 = sorted(w2.keys())
                terms2 = []
                for s in slots2:
                    terms2 += [s] * max(int(round(w2[s])), 1)
                h_in = None
                c_sum = None
                if terms2:
                    if len(terms2) == 1:
                        h_in = h_tiles[(terms2[0], l)]
                        c_sum = c_tiles[(terms2[0], l)]
                    else:
                        hot2 = (i2 in slots2)
                        eng = nc.vector if hot2 else nc.gpsimd
                        acc_h = apool.tile([128, HC * BC], F16, tag="acch")
                        acc_c = apool.tile([128, HC * BC], F16, tag="accc")
                        if hot2:
                            for kc in range(HC):
                                sl = slice(kc * BC, (kc + 1) * BC)
                                eng.tensor_add(
                                    acc_h[:, sl],
                                    h_tiles[(terms2[0], l)][:, sl],
                                    h_tiles[(terms2[1], l)][:, sl])
                                for s in terms2[2:]:
                                    eng.tensor_add(
                                        acc_h[:, sl], acc_h[:, sl],
                                        h_tiles[(s, l)][:, sl])
                            for kc in range(HC):
                                sl = slice(kc * BC, (kc + 1) * BC)
                                eng.tensor_add(
                                    acc_c[:, sl],
                                    c_tiles[(terms2[0], l)][:, sl],
                                    c_tiles[(terms2[1], l)][:, sl])
                                for s in terms2[2:]:
                                    eng.tensor_add(
                                        acc_c[:, sl], acc_c[:, sl],
                                        c_tiles[(s, l)][:, sl])
                        else:
                            eng.tensor_add(
                                acc_h[:, :], h_tiles[(terms2[0], l)][:, :],
                                h_tiles[(terms2[1], l)][:, :])
                            eng.tensor_add(
                                acc_c[:, :], c_tiles[(terms2[0], l)][:, :],
                                c_tiles[(terms2[1], l)][:, :])
                            for s in terms2[2:]:
                                eng.tensor_add(
                                    acc_h[:, :], acc_h[:, :],
                                    h_tiles[(s, l)][:, :])
                                eng.tensor_add(
                                    acc_c[:, :], acc_c[:, :],
                                    c_tiles[(s, l)][:, :])
                        h_in = acc_h
                        c_sum = acc_c
                agg_tiles[(i2, l)] = (h_in, c_sum)

            # PE warmup: ~5us of dummy matmuls during the initial weight DMA
            # wait so the HAM clock gate reaches 2.4 GHz before step 0.
            wu_src = kpool.tile([128, BC], F16, tag="wu")
            nc.vector.memset(wu_src[:, :], 0.0)
            for _ in range(3):
                wu_ps = ppool.tile([128, BC], F32, tag="gp0")
                for j in range(8):
                    nc.tensor.matmul(wu_ps[:, :], wu_src[:, 0:128],
                                     wu_src[:, :], start=(j == 0),
                                     stop=(j == 7))

            for k, i in enumerate(live_list):
                if k + 2 < len(live_list):
                    fetch_x(live_list[k + 2])
                nxt = live_list[k + 1] if k + 1 < len(live_list) else None
                w, cnt = sched[i]
                slots = sorted(w.keys())
                # expand multiplicities m_s (integer mask weights)
                terms = []
                for s in slots:
                    terms += [s] * max(int(round(w[s])), 1)
                inv = 1.0 / cnt

                # (the aggregation for this step was emitted at the end of
                # the PREVIOUS step's matching layer body via emit_agg, so
                # its adds queue right behind the producing h2/c2 and run
                # one layer earlier)
                if k == 0:
                    emit_agg(i, 0)
                    emit_agg(i, 1)

                h_l0_new = None
                vkey = int(round(cnt)) if (terms and cnt > 1.0) else 1
                for l in range(L):
                    wih = wihT0 if l == 0 else wihT1
                    whh = whh_v[(l, vkey)]
                    bias = b0 if l == 0 else b1
                    if l == 0:
                        x_chunks = [x_tiles[i][:, :]]
                    else:
                        x_chunks = [h_l0_new[:, kc * BC:(kc + 1) * BC]
                                    for kc in range(HC)]
                    h_in, c_sum = agg_tiles.pop((i, l))

                    # gate matmuls + activations per 128-wide 4H chunk, in
                    # f,f,i,i,g,g,o,o order with the DVE combine interleaved
                    # so it starts while later chunks are still on the PE.
                    gact = gpool.tile([128, NM * BC], F16, tag="gact")

                    ptag = "gp0" if l == 0 else "gp1"

                    def emit_chunk(m):
                        pre = xpre.pop((i, m), None) if l == 0 else None
                        group = []
                        if pre is None:
                            pool = ppool if l == 0 else qpool
                            ps = pool.tile([128, BC], F32, tag=ptag)
                            for kc, xch in enumerate(x_chunks):
                                group.append((wih[:, kc * G4 + m * 128:
                                                  kc * G4 + (m + 1) * 128],
                                              xch, kc == 0))
                        else:
                            ps = pre            # x-part already accumulated
                        if h_in is not None:
                            for kc in range(HC):
                                group.append((whh[:, kc * G4 + m * 128:
                                                  kc * G4 + (m + 1) * 128],
                                              h_in[:, kc * BC:(kc + 1) * BC],
                                              False))
                        for j, (lhsT, rhs, st) in enumerate(group):
                            nc.tensor.matmul(ps[:, :], lhsT, rhs,
                                             start=st,
                                             stop=(j == len(group) - 1),
                                             skip_group_check=True)
                        func = TANH if m in (4, 5) else SIG
                        nc.scalar.activation(gact[:, m * BC:(m + 1) * BC],
                                             ps[:, :], func,
                                             bias=bias[:, m:m + 1])

                    sigi = gact[:, 0 * BC:2 * BC]
                    sigf = gact[:, 2 * BC:4 * BC]
                    tg = gact[:, 4 * BC:6 * BC]
                    sigo = gact[:, 6 * BC:8 * BC]
                    col = color[i + 1]
                    c_new = spool.tile([128, HC * BC], F16, tag=f"c{col}_{l}")

                    # The combine tail runs at H-chunk-half granularity: half
                    # kc only needs o-gate chunk 6+kc and produces the half of
                    # h_new that feeds the next layer's kc-chunk matmuls.
                    th = kpool.tile([128, HC * BC], F16, tag="th")
                    h_new = spool.tile([128, HC * BC], F16, tag=f"h{col}_{l}")
                    B2 = BC                        # 512 cols per half

                    def half(ap, kc):
                        return ap[:, kc * B2:(kc + 1) * B2]

                    if c_sum is None:
                        # no predecessors: c_in = 0, so sigf is irrelevant --
                        # skip the f-gate chunks (2,3) entirely.
                        for m in (0, 1, 4, 5):
                            emit_chunk(m)
                        for kc in range(HC):
                            nc.vector.tensor_mul(half(c_new, kc),
                                                 half(sigi, kc), half(tg, kc))
                            nc.scalar.activation(half(th, kc),
                                                 half(c_new, kc), TANH)
                            emit_chunk(6 + kc)
                            nc.vector.tensor_mul(half(h_new, kc),
                                                 half(sigo, kc), half(th, kc))
                    else:
                        # f gate FIRST so the c path starts while the rest of
                        # the gate chunks are still streaming on the PE.
                        for m in (2, 3):
                            emit_chunk(m)
                        for kc in range(HC):
                            sl = slice(kc * BC, (kc + 1) * BC)
                            if cnt == 1.0:
                                nc.vector.tensor_mul(c_new[:, sl],
                                                     sigf[:, sl],
                                                     c_sum[:, sl])
                            else:
                                # c_new = (c_sum * 1/cnt) * sigf, fused
                                nc.vector.scalar_tensor_tensor(
                                    c_new[:, sl], c_sum[:, sl], inv,
                                    sigf[:, sl], ALU.mult, ALU.mult)
                        for m in (0, 1, 4, 5):     # i and g gates
                            emit_chunk(m)
                        t2 = kpool.tile([128, HC * BC], F16, tag="t2")
                        for kc in range(HC):
                            nc.vector.tensor_mul(half(t2, kc),
                                                 half(sigi, kc), half(tg, kc))
                            nc.vector.tensor_add(half(c_new, kc),
                                                 half(c_new, kc),
                                                 half(t2, kc))
                            nc.scalar.activation(half(th, kc),
                                                 half(c_new, kc), TANH)
                            emit_chunk(6 + kc)
                            nc.vector.tensor_mul(half(h_new, kc),
                                                 half(sigo, kc), half(th, kc))

                    h_tiles[(i + 1, l)] = h_new
                    c_tiles[(i + 1, l)] = c_new
                    if nxt is not None:
                        emit_agg(nxt, l)
                    if l == 0:
                        h_l0_new = h_new
                        # Software-pipelined x-projection for step i+1 layer
                        # 0: depends only on the DMA'd x tile, so the PE can
                        # run it during this step's combine tails. The psum
                        # groups stay open; step i+1's h-part matmuls join
                        # them (start=False) and close the group.
                        if nxt is not None:
                            w1 = sched[nxt][0]
                            t1list = []
                            for s1 in sorted(w1):
                                t1list += [s1] * max(int(round(w1[s1])), 1)
                            pset = (2, 3, 0, 1) if t1list else (0, 1, 4, 5)
                            for m in pset:
                                ps = ppool.tile([128, BC], F32, tag="gp0")
                                nc.tensor.matmul(
                                    ps[:, :],
                                    wihT0[:, m * 128:(m + 1) * 128],
                                    x_tiles[nxt][:, :], start=True,
                                    stop=(not t1list),
                                    skip_group_check=True)
                                xpre[(nxt, m)] = ps

            # ---- MLP ------------------------------------------------------
            hlast = h_tiles[(N, L - 1)]
            fc_chunks = [hlast[:, 0:BC], hlast[:, BC:2 * BC], featT[:, :]]

            a0 = gpool.tile([128, 4 * BC], F16, tag="gact")
            for mo in range(4):
                ps = ppool.tile([128, BC], F32, tag="gp0")
                for j, fch in enumerate(fc_chunks):
                    nc.tensor.matmul(
                        ps[:, :],
                        mw0T[:, j * MH + mo * 128: j * MH + (mo + 1) * 128],
                        fch, start=(j == 0), stop=(j == len(fc_chunks) - 1))
                nc.scalar.activation(a0[:, mo * BC:(mo + 1) * BC], ps[:, :],
                                     AF.Relu, bias=mb0[:, mo:mo + 1])

            a1 = gpool.tile([128, 4 * BC], F16, tag="gact")
            for mo in range(4):
                ps = qpool.tile([128, BC], F32, tag="gp1")
                for kc in range(4):
                    nc.tensor.matmul(
                        ps[:, :],
                        mw1T[:, kc * MH + mo * 128: kc * MH + (mo + 1) * 128],
                        a0[:, kc * BC:(kc + 1) * BC],
                        start=(kc == 0), stop=(kc == 3))
                nc.scalar.activation(a1[:, mo * BC:(mo + 1) * BC], ps[:, :],
                                     AF.Relu, bias=mb1[:, mo:mo + 1])

            psf = ppool.tile([128, BC], F32, tag="gp0")
            for kc in range(4):
                nc.tensor.matmul(psf[:1, :], mw2T[:, kc:kc + 1],
                                 a1[:, kc * BC:(kc + 1) * BC],
                                 start=(kc == 0), stop=(kc == 3))
            out_sb = kpool.tile([128, BC], F32, tag="th")
            nc.scalar.activation(out_sb[:1, :], psf[:1, :], AF.Identity,
                                 bias=mb2[:1, 0:1])
            nc.sync.dma_start(out=d_out[:, :], in_=out_sb[:1, :])

    nc.compile()
    return nc


def _prep_core_inputs(inputs):
    """Host-side layout prep shared by all cores + per-core slices."""
    f16 = np.float16
    f32 = np.float32

    wihT0 = _chunk_k(np.ascontiguousarray(inputs["Wih0"].T)).astype(f16)
    whhT0f = _chunk_k(np.ascontiguousarray(inputs["Whh0"].T))
    whhT0 = whhT0f.astype(f16)
    wihT1 = _chunk_k(np.ascontiguousarray(inputs["Wih1"].T)).astype(f16)
    whhT1f = _chunk_k(np.ascontiguousarray(inputs["Whh1"].T))
    whhT1 = whhT1f.astype(f16)

    sched = _dag_schedule(np.asarray(inputs["pred_idx"], np.int32),
                          np.asarray(inputs["pred_mask"], np.int32))
    var_arrays = {}
    for v in _needed_variants(sched):
        var_arrays[f"whhT0_v{v}"] = (whhT0f / v).astype(f16)
        var_arrays[f"whhT1_v{v}"] = (whhT1f / v).astype(f16)
    b0 = np.ascontiguousarray((inputs["bih0"] + inputs["bhh0"])
                              .astype(f32).reshape(NM, 128).T)
    b1 = np.ascontiguousarray((inputs["bih1"] + inputs["bhh1"])
                              .astype(f32).reshape(NM, 128).T)
    mw0T = _chunk_k(np.ascontiguousarray(inputs["mW0"].T)).astype(f16)
    mw1T = _chunk_k(np.ascontiguousarray(inputs["mW1"].T)).astype(f16)
    mw2T = _chunk_k(np.ascontiguousarray(inputs["mW2"].T)).astype(f16)
    mb0 = np.ascontiguousarray(inputs["mb0"].astype(f32).reshape(4, 128).T)
    mb1 = np.ascontiguousarray(inputs["mb1"].astype(f32).reshape(4, 128).T)
    mb2 = np.zeros((128, 1), f32)
    mb2[0, 0] = np.float32(inputs["mb2"][0])

    shared = dict(wihT0=wihT0, whhT0=whhT0, wihT1=wihT1, whhT1=whhT1,
                  b0=b0, b1=b1, mw0T=mw0T, mw1T=mw1T, mw2T=mw2T,
                  mb0=mb0, mb1=mb1, mb2=mb2, **var_arrays)

    dags = np.asarray(inputs["dags"], np.float32)
    feats = np.asarray(inputs["features"], np.float32)
    in_maps = []
    for c in range(NCORES):
        lo, hi = c * BC, (c + 1) * BC
        dagsT = np.ascontiguousarray(
            dags[lo:hi].transpose(1, 2, 0)).astype(f16)      # [N, IN, BC]
        featT = np.ascontiguousarray(feats[lo:hi].T).astype(f16)
        m = dict(shared)
        m["dagsT"] = dagsT
        m["featT"] = featT
        in_maps.append(m)
    return in_maps


def _get_nc(pred_idx, pred_mask):
    key = (pred_idx.tobytes(), pred_mask.tobytes())
    if key not in _BUILD_CACHE:
        _BUILD_CACHE[key] = _build(pred_idx, pred_mask)
    return _BUILD_CACHE[key]


def run(inputs, trace=False):
    from concourse.bass_utils import run_bass_kernel_spmd

    pred_idx = np.asarray(inputs["pred_idx"], np.int32)
    pred_mask = np.asarray(inputs["pred_mask"], np.int32)
    nc = _get_nc(pred_idx, pred_mask)
    in_maps = _prep_core_inputs(inputs)
    res = run_bass_kernel_spmd(nc, in_maps, core_ids=list(range(NCORES)),
                               trace=trace)
    out = np.concatenate([np.asarray(r["out"], np.float32).reshape(BC)
                          for r in res.results])
    return out, res


def kernel(**inputs) -> np.ndarray:
    out, _ = run(inputs, trace=False)
    return out


# revision 46
# speedup vs baseline: 2.3554x; 1.0517x over previous
"""DAG-LSTM + MLP Trainium2 kernel.

Data-parallel over batch: 4096 rows -> 512 per NeuronCore x 8 cores, no
collectives.  The DAG structure (pred_idx / pred_mask) is read on the host at
call time and baked into the traced instruction stream: per-step predecessor
averaging becomes a short chain of scalar_tensor_tensor ops over only the
slots that are actually written and non-zero, and slot storage in SBUF is
allocated by liveness coloring.

Layouts (everything pre-transposed on the host so the device never
transposes):
  states h/c[s][l] : SBUF [128, 2*512] fp16   col = hchunk*512 + batch
  weights W.T      : SBUF [128, nk*M]  fp16   col = kchunk*M + mcol
  gates (psum)     : PSUM [128, 512] per 128-wide 4H chunk, fp32
  gate activations : SBUF [128, 8*512] fp16  (i,i,f,f,g,g,o,o chunk order)

Engines: PE fp16 matmuls (1 cyc/row, split psum rings per layer, software-
pipelined x-projections one step ahead, warmup spin for the HAM clock gate);
ACT sigmoid/tanh with the (bih+bhh) bias folded in, f-gate chunks first so
the c path starts early (f skipped entirely on no-predecessor steps); DVE
runs the combine tail at H-chunk-half granularity (fp16 2x mode); GPSIMD
does the unscaled predecessor sums (1/cnt lives in host-prescaled Whh/cnt
copies and the fused scalar_tensor_tensor), except on steps that read the
immediately-previous slot, which aggregate on DVE to shorten the critical
chain.
"""

import os
import sys

import numpy as np

for _p in ("/opt/trn_rl_repo",):
    if _p not in sys.path and os.path.isdir(_p):
        sys.path.insert(0, _p)

B, N, P = 4096, 24, 3
IN, H, L = 128, 256, 2
EXTRA, MH, OUT = 128, 512, 1
NCORES = 8
BC = B // NCORES            # 512 batch rows per core
G4 = 4 * H                  # 1024 gate width
NM = G4 // 128              # 8 gate chunks of 128
HC = H // 128               # 2 hidden chunks

_BUILD_CACHE = {}


def _chunk_k(wt: np.ndarray) -> np.ndarray:
    """[K, M] -> [128, (K//128)*M] with col = kchunk*M + m."""
    k, m = wt.shape
    nk = k // 128
    assert nk * 128 == k
    return np.ascontiguousarray(wt.reshape(nk, 128, m).transpose(1, 0, 2).reshape(128, nk * m))


def _dag_schedule(pred_idx: np.ndarray, pred_mask: np.ndarray):
    """Per step: (weights {slot: w/cnt for available non-zero slots}).

    Matches the reference exactly for arbitrary pred_idx/pred_mask: slot 0 and
    slots not yet written at step i read as zeros (dropped from the sum), but
    every mask unit still counts toward cnt.
    """
    sched = []
    for i in range(N):
        tot = 0.0
        w = {}
        for p in range(P):
            m = float(pred_mask[i, p])
            if m == 0.0:
                continue
            tot += m
            s = int(pred_idx[i, p])
            if 1 <= s <= i:                      # written and non-zero slot
                w[s] = w.get(s, 0.0) + m
        cnt = max(tot, 1.0)
        sched.append((w, cnt))
    return sched


def _needed_variants(sched):
    """cnt values > 1 that occur on steps with at least one live pred slot.
    For those steps the h predecessor sum is left UNSCALED and the matmul
    uses a host-prescaled Whh/cnt copy instead."""
    out = set()
    for w, cnt in sched:
        if w and cnt > 1.0:
            out.add(int(round(cnt)))
    return sorted(out)


def _live_steps(sched):
    """Backward reachability from the output node: only steps whose slot is
    (transitively) read on the path to slot N (the MLP input) need to run.
    Skipping the rest is exact -- their values are unobservable."""
    live = set()
    stack = [N - 1]
    while stack:
        i = stack.pop()
        if i in live:
            continue
        live.add(i)
        for s in sched[i][0]:          # live pred slots; producer = step s-1
            if s - 1 not in live:
                stack.append(s - 1)
    return live


def _color_slots(sched):
    """Greedy interval coloring of slots 1..N. Slot s is born at step s-1 and
    last read at max step using it (slot N also read by the final MLP)."""
    last = {}
    for i, (w, _cnt) in enumerate(sched):
        for s in w:
            last[s] = i
    last[N] = max(last.get(N, 0), N)             # final MLP reads h[N][1]
    color = {}
    free = []
    ncol = 0
    active = []                                   # (last_use, slot)
    for s in range(1, N + 1):
        born = s - 1
        still = []
        for lu, sl in active:
            if lu < born:
                free.append(color[sl])
            else:
                still.append((lu, sl))
        active = still
        if free:
            c = free.pop()
        else:
            c = ncol
            ncol += 1
        color[s] = c
        active.append((last.get(s, born - 1), s))
    return color, ncol, last


def _build(pred_idx: np.ndarray, pred_mask: np.ndarray):
    import concourse.bacc as bacc
    import concourse.tile as tile
    import concourse.mybir as mybir

    F16 = mybir.dt.float16
    F32 = mybir.dt.float32
    AF = mybir.ActivationFunctionType
    ALU = mybir.AluOpType

    sched = _dag_schedule(pred_idx, pred_mask)
    color, ncol, _last = _color_slots(sched)
    live_list = sorted(_live_steps(sched))
    prev_live = {live_list[k]: live_list[k - 1]
                 for k in range(1, len(live_list))}

    nc = bacc.Bacc("TRN2", target_bir_lowering=False, debug=False,
                   enable_asserts=False, num_devices=NCORES)

    # ---- DRAM parameters (per-core, preprocessed on host) -------------------
    d_dagsT = nc.dram_tensor("dagsT", [N, IN, BC], F16, kind="ExternalInput")
    d_featT = nc.dram_tensor("featT", [EXTRA, BC], F16, kind="ExternalInput")
    d_wihT0 = nc.dram_tensor("wihT0", [128, G4], F16, kind="ExternalInput")
    d_whhT0 = nc.dram_tensor("whhT0", [128, HC * G4], F16, kind="ExternalInput")
    d_wihT1 = nc.dram_tensor("wihT1", [128, HC * G4], F16, kind="ExternalInput")
    d_whhT1 = nc.dram_tensor("whhT1", [128, HC * G4], F16, kind="ExternalInput")
    variants = _needed_variants(sched)
    d_whh_v = {}
    for v in variants:
        for l in range(L):
            d_whh_v[(l, v)] = nc.dram_tensor(
                f"whhT{l}_v{v}", [128, HC * G4], F16, kind="ExternalInput")
    d_b0 = nc.dram_tensor("b0", [128, NM], F32, kind="ExternalInput")
    d_b1 = nc.dram_tensor("b1", [128, NM], F32, kind="ExternalInput")
    d_mw0T = nc.dram_tensor("mw0T", [128, 3 * MH], F16, kind="ExternalInput")
    d_mw1T = nc.dram_tensor("mw1T", [128, 4 * MH], F16, kind="ExternalInput")
    d_mw2T = nc.dram_tensor("mw2T", [128, 4], F16, kind="ExternalInput")
    d_mb0 = nc.dram_tensor("mb0", [128, 4], F32, kind="ExternalInput")
    d_mb1 = nc.dram_tensor("mb1", [128, 4], F32, kind="ExternalInput")
    d_mb2 = nc.dram_tensor("mb2", [128, 1], F32, kind="ExternalInput")
    d_out = nc.dram_tensor("out", [1, BC], F32, kind="ExternalOutput")

    with tile.TileContext(nc) as tc:
        from contextlib import ExitStack
        with ExitStack() as ctx:
            wpool = ctx.enter_context(tc.tile_pool(name="weights", bufs=1))
            spool = ctx.enter_context(tc.tile_pool(name="states", bufs=1))
            # high-color DAGs need the SBUF back for state tiles
            gpool = ctx.enter_context(
                tc.tile_pool(name="gact", bufs=4 if ncol <= 11 else 2))
            xpool = ctx.enter_context(tc.tile_pool(name="xin", bufs=3))
            kpool = ctx.enter_context(tc.tile_pool(name="work", bufs=3))
            apool = ctx.enter_context(tc.tile_pool(name="agg", bufs=4))
            ppool = ctx.enter_context(tc.tile_pool(name="psum", bufs=6, space="PSUM"))
            qpool = ctx.enter_context(tc.tile_pool(name="psum1", bufs=2, space="PSUM"))

            # ---- load weights ----------------------------------------------
            def wload(tag, dram, shape, dt):
                t = wpool.tile(shape, dt, tag=tag)
                nc.sync.dma_start(out=t[:, :], in_=dram[:, :])
                return t

            x_tiles = {}

            def fetch_x(i):
                if i < N and i not in x_tiles:
                    t = xpool.tile([128, BC], F16, tag="x")
                    nc.sync.dma_start(out=t[:, :], in_=d_dagsT[i])
                    x_tiles[i] = t

            # step-0 critical path first, then the rest
            wihT0 = wload("wihT0", d_wihT0, [128, G4], F16)
            b0 = wload("b0", d_b0, [128, NM], F32)
            fetch_x(live_list[0])
            whhT0 = wload("whhT0", d_whhT0, [128, HC * G4], F16)
            if len(live_list) > 1:
                fetch_x(live_list[1])
            wihT1 = wload("wihT1", d_wihT1, [128, HC * G4], F16)
            whhT1 = wload("whhT1", d_whhT1, [128, HC * G4], F16)
            b1 = wload("b1", d_b1, [128, NM], F32)
            whh_v = {(0, 1): whhT0, (1, 1): whhT1}
            for (l, v), dram in d_whh_v.items():
                whh_v[(l, v)] = wload(f"whhT{l}_v{v}", dram,
                                      [128, HC * G4], F16)
            featT = wload("featT", d_featT, [EXTRA, BC], F16)
            mw0T = wload("mw0T", d_mw0T, [128, 3 * MH], F16)
            mw1T = wload("mw1T", d_mw1T, [128, 4 * MH], F16)
            mw2T = wload("mw2T", d_mw2T, [128, 4], F16)
            mb0 = wload("mb0", d_mb0, [128, 4], F32)
            mb1 = wload("mb1", d_mb1, [128, 4], F32)
            mb2 = wload("mb2", d_mb2, [128, 1], F32)

            h_tiles = {}                           # (slot, layer) -> tile
            c_tiles = {}
            xpre = {}                              # (step, chunk) -> open psum
            agg_tiles = {}                         # (step, layer) -> (h, c)

            SIG = AF.Sigmoid
            TANH = AF.Tanh

            def emit_agg(i2, l):
                """Predecessor aggregation for (step i2, layer l), emitted as
                soon as this layer's last needed slot exists. GPSIMD only
                implements plain Add/Multiply, so the sums are UNSCALED; the
                1/cnt scale lives in the prescaled Whh/cnt weights (h path)
                and the fused scalar_tensor_tensor (c path). Steps that read
                the slot written one step earlier are on the critical path:
                those aggregate on DVE, half-split, right behind h2."""
                w2, _c2 = sched[i2]
                pl = prev_live.get(i2, -1)
                # a slot produced by the immediately-preceding LIVE step
                # arrives last -> that agg is on the critical path
                slots2 = sorted(w2.keys(), key=lambda s: (s - 1 == pl, s))
                terms2 = []
                for s in slots2:
                    terms2 += [s] * max(int(round(w2[s])), 1)
                h_in = None
                c_sum = None
                if terms2:
                    if len(terms2) == 1:
                        h_in = h_tiles[(terms2[0], l)]
                        c_sum = c_tiles[(terms2[0], l)]
                    else:
                        hot2 = any(s - 1 == pl for s in slots2)
                        eng = nc.vector if hot2 else nc.gpsimd
                        acc_h = apool.tile([128, HC * BC], F16, tag="acch")
                        acc_c = apool.tile([128, HC * BC], F16, tag="accc")
                        if hot2:
                            for kc in range(HC):
                                sl = slice(kc * BC, (kc + 1) * BC)
                                eng.tensor_add(
                                    acc_h[:, sl],
                                    h_tiles[(terms2[0], l)][:, sl],
                                    h_tiles[(terms2[1], l)][:, sl])
                                for s in terms2[2:]:
                                    eng.tensor_add(
                                        acc_h[:, sl], acc_h[:, sl],
                                        h_tiles[(s, l)][:, sl])
                            for kc in range(HC):
                                sl = slice(kc * BC, (kc + 1) * BC)
                                eng.tensor_add(
                                    acc_c[:, sl],
                                    c_tiles[(terms2[0], l)][:, sl],
                                    c_tiles[(terms2[1], l)][:, sl])
                                for s in terms2[2:]:
                                    eng.tensor_add(
                                        acc_c[:, sl], acc_c[:, sl],
                                        c_tiles[(s, l)][:, sl])
                        else:
                            eng.tensor_add(
                                acc_h[:, :], h_tiles[(terms2[0], l)][:, :],
                                h_tiles[(terms2[1], l)][:, :])
                            eng.tensor_add(
                                acc_c[:, :], c_tiles[(terms2[0], l)][:, :],
                                c_tiles[(terms2[1], l)][:, :])
                            for s in terms2[2:]:
                                eng.tensor_add(
                                    acc_h[:, :], acc_h[:, :],
                                    h_tiles[(s, l)][:, :])
                                eng.tensor_add(
                                    acc_c[:, :], acc_c[:, :],
                                    c_tiles[(s, l)][:, :])
                        h_in = acc_h
                        c_sum = acc_c
                agg_tiles[(i2, l)] = (h_in, c_sum)

            # PE warmup: ~5us of dummy matmuls during the initial weight DMA
            # wait so the HAM clock gate reaches 2.4 GHz before step 0.
            wu_src = kpool.tile([128, BC], F16, tag="wu")
            nc.vector.memset(wu_src[:, :], 0.0)
            for _ in range(3):
                wu_ps = ppool.tile([128, BC], F32, tag="gp0")
                for j in range(8):
                    nc.tensor.matmul(wu_ps[:, :], wu_src[:, 0:128],
                                     wu_src[:, :], start=(j == 0),
                                     stop=(j == 7))

            for k, i in enumerate(live_list):
                if k + 2 < len(live_list):
                    fetch_x(live_list[k + 2])
                nxt = live_list[k + 1] if k + 1 < len(live_list) else None
                w, cnt = sched[i]
                slots = sorted(w.keys())
                # expand multiplicities m_s (integer mask weights)
                terms = []
                for s in slots:
                    terms += [s] * max(int(round(w[s])), 1)
                inv = 1.0 / cnt

                # (the aggregation for this step was emitted at the end of
                # the PREVIOUS step's matching layer body via emit_agg, so
                # its adds queue right behind the producing h2/c2 and run
                # one layer earlier)
                if k == 0:
                    emit_agg(i, 0)
                    emit_agg(i, 1)

                h_l0_new = None
                vkey = int(round(cnt)) if (terms and cnt > 1.0) else 1
                for l in range(L):
                    wih = wihT0 if l == 0 else wihT1
                    whh = whh_v[(l, vkey)]
                    bias = b0 if l == 0 else b1
                    if l == 0:
                        x_chunks = [x_tiles[i][:, :]]
                    else:
                        x_chunks = [h_l0_new[:, kc * BC:(kc + 1) * BC]
                                    for kc in range(HC)]
                    h_in, c_sum = agg_tiles.pop((i, l))

                    # gate matmuls + activations per 128-wide 4H chunk, in
                    # f,f,i,i,g,g,o,o order with the DVE combine interleaved
                    # so it starts while later chunks are still on the PE.
                    gact = gpool.tile([128, NM * BC], F16, tag="gact")

                    ptag = "gp0" if l == 0 else "gp1"

                    def emit_chunk(m):
                        pre = xpre.pop((i, m), None) if l == 0 else None
                        group = []
                        if pre is None:
                            pool = ppool if l == 0 else qpool
                            ps = pool.tile([128, BC], F32, tag=ptag)
                            for kc, xch in enumerate(x_chunks):
                                group.append((wih[:, kc * G4 + m * 128:
                                                  kc * G4 + (m + 1) * 128],
                                              xch, kc == 0))
                        else:
                            ps = pre            # x-part already accumulated
                        if h_in is not None:
                            for kc in range(HC):
                                group.append((whh[:, kc * G4 + m * 128:
                                                  kc * G4 + (m + 1) * 128],
                                              h_in[:, kc * BC:(kc + 1) * BC],
                                              False))
                        for j, (lhsT, rhs, st) in enumerate(group):
                            nc.tensor.matmul(ps[:, :], lhsT, rhs,
                                             start=st,
                                             stop=(j == len(group) - 1),
                                             skip_group_check=True)
                        func = TANH if m in (4, 5) else SIG
                        nc.scalar.activation(gact[:, m * BC:(m + 1) * BC],
                                             ps[:, :], func,
                                             bias=bias[:, m:m + 1])

                    sigi = gact[:, 0 * BC:2 * BC]
                    sigf = gact[:, 2 * BC:4 * BC]
                    tg = gact[:, 4 * BC:6 * BC]
                    sigo = gact[:, 6 * BC:8 * BC]
                    col = color[i + 1]
                    c_new = spool.tile([128, HC * BC], F16, tag=f"c{col}_{l}")

                    # The combine tail runs at H-chunk-half granularity: half
                    # kc only needs o-gate chunk 6+kc and produces the half of
                    # h_new that feeds the next layer's kc-chunk matmuls.
                    th = kpool.tile([128, HC * BC], F16, tag="th")
                    h_new = spool.tile([128, HC * BC], F16, tag=f"h{col}_{l}")
                    B2 = BC                        # 512 cols per half

                    def half(ap, kc):
                        return ap[:, kc * B2:(kc + 1) * B2]

                    if c_sum is None:
                        # no predecessors: c_in = 0, so sigf is irrelevant --
                        # skip the f-gate chunks (2,3) entirely.
                        for m in (0, 1, 4, 5):
                            emit_chunk(m)
                        for kc in range(HC):
                            nc.vector.tensor_mul(half(c_new, kc),
                                                 half(sigi, kc), half(tg, kc))
                            nc.scalar.activation(half(th, kc),
                                                 half(c_new, kc), TANH)
                            emit_chunk(6 + kc)
                            nc.vector.tensor_mul(half(h_new, kc),
                                                 half(sigo, kc), half(th, kc))
                    else:
                        # f gate FIRST so the c path starts while the rest of
                        # the gate chunks are still streaming on the PE.
                        for m in (2, 3):
                            emit_chunk(m)
                        for kc in range(HC):
                            sl = slice(kc * BC, (kc + 1) * BC)
                            if cnt == 1.0:
                                nc.vector.tensor_mul(c_new[:, sl],
                                                     sigf[:, sl],
                                                     c_sum[:, sl])
                            else:
                                # c_new = (c_sum * 1/cnt) * sigf, fused
                                nc.vector.scalar_tensor_tensor(
                                    c_new[:, sl], c_sum[:, sl], inv,
                                    sigf[:, sl], ALU.mult, ALU.mult)
                        for m in (0, 1, 4, 5):     # i and g gates
                            emit_chunk(m)
                        t2 = kpool.tile([128, HC * BC], F16, tag="t2")
                        for kc in range(HC):
                            nc.vector.tensor_mul(half(t2, kc),
                                                 half(sigi, kc), half(tg, kc))
                            nc.vector.tensor_add(half(c_new, kc),
                                                 half(c_new, kc),
                                                 half(t2, kc))
                            nc.scalar.activation(half(th, kc),
                                                 half(c_new, kc), TANH)
                            emit_chunk(6 + kc)
                            nc.vector.tensor_mul(half(h_new, kc),
                                                 half(sigo, kc), half(th, kc))

                    h_tiles[(i + 1, l)] = h_new
                    c_tiles[(i + 1, l)] = c_new
                    if nxt is not None:
                        emit_agg(nxt, l)
                    if l == 0:
                        h_l0_new = h_new
                        # Software-pipelined x-projection for step i+1 layer
                        # 0: depends only on the DMA'd x tile, so the PE can
                        # run it during this step's combine tails. The psum
                        # groups stay open; step i+1's h-part matmuls join
                        # them (start=False) and close the group.
                        if nxt is not None:
                            w1 = sched[nxt][0]
                            t1list = []
                            for s1 in sorted(w1):
                                t1list += [s1] * max(int(round(w1[s1])), 1)
                            pset = (2, 3, 0, 1) if t1list else (0, 1, 4, 5)
                            for m in pset:
                                ps = ppool.tile([128, BC], F32, tag="gp0")
                                nc.tensor.matmul(
                                    ps[:, :],
                                    wihT0[:, m * 128:(m + 1) * 128],
                                    x_tiles[nxt][:, :], start=True,
                                    stop=(not t1list),
                                    skip_group_check=True)
                                xpre[(nxt, m)] = ps

            # ---- MLP ------------------------------------------------------
            hlast = h_tiles[(N, L - 1)]
            fc_chunks = [hlast[:, 0:BC], hlast[:, BC:2 * BC], featT[:, :]]

            a0 = gpool.tile([128, 4 * BC], F16, tag="gact")
            for mo in range(4):
                ps = ppool.tile([128, BC], F32, tag="gp0")
                for j, fch in enumerate(fc_chunks):
                    nc.tensor.matmul(
                        ps[:, :],
                        mw0T[:, j * MH + mo * 128: j * MH + (mo + 1) * 128],
                        fch, start=(j == 0), stop=(j == len(fc_chunks) - 1))
                nc.scalar.activation(a0[:, mo * BC:(mo + 1) * BC], ps[:, :],
                                     AF.Relu, bias=mb0[:, mo:mo + 1])

            a1 = gpool.tile([128, 4 * BC], F16, tag="gact")
            for mo in range(4):
                ps = qpool.tile([128, BC], F32, tag="gp1")
                for kc in range(4):
                    nc.tensor.matmul(
                        ps[:, :],
                        mw1T[:, kc * MH + mo * 128: kc * MH + (mo + 1) * 128],
                        a0[:, kc * BC:(kc + 1) * BC],
                        start=(kc == 0), stop=(kc == 3))
                nc.scalar.activation(a1[:, mo * BC:(mo + 1) * BC], ps[:, :],
                                     AF.Relu, bias=mb1[:, mo:mo + 1])

            psf = ppool.tile([128, BC], F32, tag="gp0")
            for kc in range(4):
                nc.tensor.matmul(psf[:1, :], mw2T[:, kc:kc + 1],
                                 a1[:, kc * BC:(kc + 1) * BC],
                                 start=(kc == 0), stop=(kc == 3))
            out_sb = kpool.tile([128, BC], F32, tag="th")
            nc.scalar.activation(out_sb[:1, :], psf[:1, :], AF.Identity,
                                 bias=mb2[:1, 0:1])
            nc.sync.dma_start(out=d_out[:, :], in_=out_sb[:1, :])

    nc.compile()
    return nc


def _prep_core_inputs(inputs):
    """Host-side layout prep shared by all cores + per-core slices."""
    f16 = np.float16
    f32 = np.float32

    wihT0 = _chunk_k(np.ascontiguousarray(inputs["Wih0"].T)).astype(f16)
    whhT0f = _chunk_k(np.ascontiguousarray(inputs["Whh0"].T))
    whhT0 = whhT0f.astype(f16)
    wihT1 = _chunk_k(np.ascontiguousarray(inputs["Wih1"].T)).astype(f16)
    whhT1f = _chunk_k(np.ascontiguousarray(inputs["Whh1"].T))
    whhT1 = whhT1f.astype(f16)

    sched = _dag_schedule(np.asarray(inputs["pred_idx"], np.int32),
                          np.asarray(inputs["pred_mask"], np.int32))
    var_arrays = {}
    for v in _needed_variants(sched):
        var_arrays[f"whhT0_v{v}"] = (whhT0f / v).astype(f16)
        var_arrays[f"whhT1_v{v}"] = (whhT1f / v).astype(f16)
    b0 = np.ascontiguousarray((inputs["bih0"] + inputs["bhh0"])
                              .astype(f32).reshape(NM, 128).T)
    b1 = np.ascontiguousarray((inputs["bih1"] + inputs["bhh1"])
                              .astype(f32).reshape(NM, 128).T)
    mw0T = _chunk_k(np.ascontiguousarray(inputs["mW0"].T)).astype(f16)
    mw1T = _chunk_k(np.ascontiguousarray(inputs["mW1"].T)).astype(f16)
    mw2T = _chunk_k(np.ascontiguousarray(inputs["mW2"].T)).astype(f16)
    mb0 = np.ascontiguousarray(inputs["mb0"].astype(f32).reshape(4, 128).T)
    mb1 = np.ascontiguousarray(inputs["mb1"].astype(f32).reshape(4, 128).T)
    mb2 = np.zeros((128, 1), f32)
    mb2[0, 0] = np.float32(inputs["mb2"][0])

    shared = dict(wihT0=wihT0, whhT0=whhT0, wihT1=wihT1, whhT1=whhT1,
                  b0=b0, b1=b1, mw0T=mw0T, mw1T=mw1T, mw2T=mw2T,
                  mb0=mb0, mb1=mb1, mb2=mb2, **var_arrays)

    dags = np.asarray(inputs["dags"], np.float32)
    feats = np.asarray(inputs["features"], np.float32)
    in_maps = []
    for c in range(NCORES):
        lo, hi = c * BC, (c + 1) * BC
        dagsT = np.ascontiguousarray(
            dags[lo:hi].transpose(1, 2, 0)).astype(f16)      # [N, IN, BC]
        featT = np.ascontiguousarray(feats[lo:hi].T).astype(f16)
        m = dict(shared)
        m["dagsT"] = dagsT
        m["featT"] = featT
        in_maps.append(m)
    return in_maps


def _get_nc(pred_idx, pred_mask):
    key = (pred_idx.tobytes(), pred_mask.tobytes())
    if key not in _BUILD_CACHE:
        _BUILD_CACHE[key] = _build(pred_idx, pred_mask)
    return _BUILD_CACHE[key]


def run(inputs, trace=False):
    from concourse.bass_utils import run_bass_kernel_spmd

    pred_idx = np.asarray(inputs["pred_idx"], np.int32)
    pred_mask = np.asarray(inputs["pred_mask"], np.int32)
    nc = _get_nc(pred_idx, pred_mask)
    in_maps = _prep_core_inputs(inputs)
    res = run_bass_kernel_spmd(nc, in_maps, core_ids=list(range(NCORES)),
                               trace=trace)
    out = np.concatenate([np.asarray(r["out"], np.float32).reshape(BC)
                          for r in res.results])
    return out, res


def kernel(**inputs) -> np.ndarray:
    out, _ = run(inputs, trace=False)
    return out


# revision 47
# speedup vs baseline: 2.3720x; 1.0070x over previous
"""DAG-LSTM + MLP Trainium2 kernel.

Data-parallel over batch: 4096 rows -> 512 per NeuronCore x 8 cores, no
collectives.  The DAG structure (pred_idx / pred_mask) is read on the host at
call time and baked into the traced instruction stream: per-step predecessor
averaging becomes a short chain of scalar_tensor_tensor ops over only the
slots that are actually written and non-zero, and slot storage in SBUF is
allocated by liveness coloring.

Layouts (everything pre-transposed on the host so the device never
transposes):
  states h/c[s][l] : SBUF [128, 2*512] fp16   col = hchunk*512 + batch
  weights W.T      : SBUF [128, nk*M]  fp16   col = kchunk*M + mcol
  gates (psum)     : PSUM [128, 512] per 128-wide 4H chunk, fp32
  gate activations : SBUF [128, 8*512] fp16  (i,i,f,f,g,g,o,o chunk order)

Engines: PE fp16 matmuls (1 cyc/row, split psum rings per layer, software-
pipelined x-projections one step ahead, warmup spin for the HAM clock gate);
ACT sigmoid/tanh with the (bih+bhh) bias folded in, f-gate chunks first so
the c path starts early (f skipped entirely on no-predecessor steps); DVE
runs the combine tail at H-chunk-half granularity (fp16 2x mode); GPSIMD
does the unscaled predecessor sums (1/cnt lives in host-prescaled Whh/cnt
copies and the fused scalar_tensor_tensor), except on steps that read the
immediately-previous slot, which aggregate on DVE to shorten the critical
chain.
"""

import os
import sys

import numpy as np

for _p in ("/opt/trn_rl_repo",):
    if _p not in sys.path and os.path.isdir(_p):
        sys.path.insert(0, _p)

B, N, P = 4096, 24, 3
IN, H, L = 128, 256, 2
EXTRA, MH, OUT = 128, 512, 1
NCORES = 8
BC = B // NCORES            # 512 batch rows per core
G4 = 4 * H                  # 1024 gate width
NM = G4 // 128              # 8 gate chunks of 128
HC = H // 128               # 2 hidden chunks

_BUILD_CACHE = {}


def _chunk_k(wt: np.ndarray) -> np.ndarray:
    """[K, M] -> [128, (K//128)*M] with col = kchunk*M + m."""
    k, m = wt.shape
    nk = k // 128
    assert nk * 128 == k
    return np.ascontiguousarray(wt.reshape(nk, 128, m).transpose(1, 0, 2).reshape(128, nk * m))


def _dag_schedule(pred_idx: np.ndarray, pred_mask: np.ndarray):
    """Per step: (weights {slot: w/cnt for available non-zero slots}).

    Matches the reference exactly for arbitrary pred_idx/pred_mask: slot 0 and
    slots not yet written at step i read as zeros (dropped from the sum), but
    every mask unit still counts toward cnt.
    """
    sched = []
    for i in range(N):
        tot = 0.0
        w = {}
        for p in range(P):
            m = float(pred_mask[i, p])
            if m == 0.0:
                continue
            tot += m
            s = int(pred_idx[i, p])
            if 1 <= s <= i:                      # written and non-zero slot
                w[s] = w.get(s, 0.0) + m
        cnt = max(tot, 1.0)
        sched.append((w, cnt))
    return sched


def _needed_variants(sched):
    """cnt values > 1 that occur on steps with at least one live pred slot.
    For those steps the h predecessor sum is left UNSCALED and the matmul
    uses a host-prescaled Whh/cnt copy instead."""
    out = set()
    for w, cnt in sched:
        if w and cnt > 1.0:
            out.add(int(round(cnt)))
    return sorted(out)


def _live_steps(sched):
    """Backward reachability from the output node: only steps whose slot is
    (transitively) read on the path to slot N (the MLP input) need to run.
    Skipping the rest is exact -- their values are unobservable."""
    live = set()
    stack = [N - 1]
    while stack:
        i = stack.pop()
        if i in live:
            continue
        live.add(i)
        for s in sched[i][0]:          # live pred slots; producer = step s-1
            if s - 1 not in live:
                stack.append(s - 1)
    return live


def _color_slots(sched):
    """Greedy interval coloring of slots 1..N. Slot s is born at step s-1 and
    last read at max step using it (slot N also read by the final MLP)."""
    last = {}
    for i, (w, _cnt) in enumerate(sched):
        for s in w:
            last[s] = i
    last[N] = max(last.get(N, 0), N)             # final MLP reads h[N][1]
    color = {}
    free = []
    ncol = 0
    active = []                                   # (last_use, slot)
    for s in range(1, N + 1):
        born = s - 1
        still = []
        for lu, sl in active:
            if lu < born:
                free.append(color[sl])
            else:
                still.append((lu, sl))
        active = still
        if free:
            c = free.pop()
        else:
            c = ncol
            ncol += 1
        color[s] = c
        active.append((last.get(s, born - 1), s))
    return color, ncol, last


def _build(pred_idx: np.ndarray, pred_mask: np.ndarray):
    import concourse.bacc as bacc
    import concourse.tile as tile
    import concourse.mybir as mybir

    F16 = mybir.dt.float16
    F32 = mybir.dt.float32
    AF = mybir.ActivationFunctionType
    ALU = mybir.AluOpType

    sched = _dag_schedule(pred_idx, pred_mask)
    color, ncol, _last = _color_slots(sched)
    live_list = sorted(_live_steps(sched))
    prev_live = {live_list[k]: live_list[k - 1]
                 for k in range(1, len(live_list))}

    nc = bacc.Bacc("TRN2", target_bir_lowering=False, debug=False,
                   enable_asserts=False, num_devices=NCORES)

    # ---- DRAM parameters (per-core, preprocessed on host) -------------------
    d_dagsT = nc.dram_tensor("dagsT", [N, IN, BC], F16, kind="ExternalInput")
    d_featT = nc.dram_tensor("featT", [EXTRA, BC], F16, kind="ExternalInput")
    d_wihT0 = nc.dram_tensor("wihT0", [128, G4], F16, kind="ExternalInput")
    d_whhT0 = nc.dram_tensor("whhT0", [128, HC * G4], F16, kind="ExternalInput")
    d_wihT1 = nc.dram_tensor("wihT1", [128, HC * G4], F16, kind="ExternalInput")
    d_whhT1 = nc.dram_tensor("whhT1", [128, HC * G4], F16, kind="ExternalInput")
    variants = _needed_variants(sched)
    d_whh_v = {}
    for v in variants:
        for l in range(L):
            d_whh_v[(l, v)] = nc.dram_tensor(
                f"whhT{l}_v{v}", [128, HC * G4], F16, kind="ExternalInput")
    d_b0 = nc.dram_tensor("b0", [128, NM], F32, kind="ExternalInput")
    d_b1 = nc.dram_tensor("b1", [128, NM], F32, kind="ExternalInput")
    d_mw0T = nc.dram_tensor("mw0T", [128, 3 * MH], F16, kind="ExternalInput")
    d_mw1T = nc.dram_tensor("mw1T", [128, 4 * MH], F16, kind="ExternalInput")
    d_mw2T = nc.dram_tensor("mw2T", [128, 4], F16, kind="ExternalInput")
    d_mb0 = nc.dram_tensor("mb0", [128, 4], F32, kind="ExternalInput")
    d_mb1 = nc.dram_tensor("mb1", [128, 4], F32, kind="ExternalInput")
    d_mb2 = nc.dram_tensor("mb2", [128, 1], F32, kind="ExternalInput")
    d_out = nc.dram_tensor("out", [1, BC], F32, kind="ExternalOutput")

    with tile.TileContext(nc) as tc:
        from contextlib import ExitStack
        with ExitStack() as ctx:
            wpool = ctx.enter_context(tc.tile_pool(name="weights", bufs=1))
            spool = ctx.enter_context(tc.tile_pool(name="states", bufs=1))
            # high-color DAGs need the SBUF back for state tiles
            gpool = ctx.enter_context(
                tc.tile_pool(name="gact", bufs=4 if ncol <= 11 else 2))
            xpool = ctx.enter_context(tc.tile_pool(name="xin", bufs=3))
            kpool = ctx.enter_context(tc.tile_pool(name="work", bufs=3))
            apool = ctx.enter_context(tc.tile_pool(name="agg", bufs=4))
            ppool = ctx.enter_context(tc.tile_pool(name="psum", bufs=6, space="PSUM"))
            qpool = ctx.enter_context(tc.tile_pool(name="psum1", bufs=2, space="PSUM"))

            # ---- load weights ----------------------------------------------
            def wload(tag, dram, shape, dt, split=1):
                t = wpool.tile(shape, dt, tag=tag)
                step = shape[1] // split
                for j in range(split):
                    nc.sync.dma_start(out=t[:, j * step:(j + 1) * step],
                                      in_=dram[:, j * step:(j + 1) * step])
                return t

            x_tiles = {}

            def fetch_x(i):
                if i < N and i not in x_tiles:
                    t = xpool.tile([128, BC], F16, tag="x")
                    nc.sync.dma_start(out=t[:, :], in_=d_dagsT[i])
                    x_tiles[i] = t

            # step-0 critical path first, then the rest
            wihT0 = wload("wihT0", d_wihT0, [128, G4], F16, split=2)
            b0 = wload("b0", d_b0, [128, NM], F32)
            fetch_x(live_list[0])
            whhT0 = wload("whhT0", d_whhT0, [128, HC * G4], F16, split=2)
            if len(live_list) > 1:
                fetch_x(live_list[1])
            wihT1 = wload("wihT1", d_wihT1, [128, HC * G4], F16, split=2)
            whhT1 = wload("whhT1", d_whhT1, [128, HC * G4], F16, split=2)
            b1 = wload("b1", d_b1, [128, NM], F32)
            whh_v = {(0, 1): whhT0, (1, 1): whhT1}
            for (l, v), dram in d_whh_v.items():
                whh_v[(l, v)] = wload(f"whhT{l}_v{v}", dram,
                                      [128, HC * G4], F16)
            featT = wload("featT", d_featT, [EXTRA, BC], F16)
            mw0T = wload("mw0T", d_mw0T, [128, 3 * MH], F16)
            mw1T = wload("mw1T", d_mw1T, [128, 4 * MH], F16)
            mw2T = wload("mw2T", d_mw2T, [128, 4], F16)
            mb0 = wload("mb0", d_mb0, [128, 4], F32)
            mb1 = wload("mb1", d_mb1, [128, 4], F32)
            mb2 = wload("mb2", d_mb2, [128, 1], F32)

            h_tiles = {}                           # (slot, layer) -> tile
            c_tiles = {}
            xpre = {}                              # (step, chunk) -> open psum
            agg_tiles = {}                         # (step, layer) -> (h, c)

            SIG = AF.Sigmoid
            TANH = AF.Tanh

            def emit_agg(i2, l):
                """Predecessor aggregation for (step i2, layer l), emitted as
                soon as this layer's last needed slot exists. GPSIMD only
                implements plain Add/Multiply, so the sums are UNSCALED; the
                1/cnt scale lives in the prescaled Whh/cnt weights (h path)
                and the fused scalar_tensor_tensor (c path). Steps that read
                the slot written one step earlier are on the critical path:
                those aggregate on DVE, half-split, right behind h2."""
                w2, _c2 = sched[i2]
                pl = prev_live.get(i2, -1)
                # a slot produced by the immediately-preceding LIVE step
                # arrives last -> that agg is on the critical path
                slots2 = sorted(w2.keys(), key=lambda s: (s - 1 == pl, s))
                terms2 = []
                for s in slots2:
                    terms2 += [s] * max(int(round(w2[s])), 1)
                h_in = None
                c_sum = None
                if terms2:
                    if len(terms2) == 1:
                        h_in = h_tiles[(terms2[0], l)]
                        c_sum = c_tiles[(terms2[0], l)]
                    else:
                        hot2 = any(s - 1 == pl for s in slots2)
                        eng = nc.vector if hot2 else nc.gpsimd
                        acc_h = apool.tile([128, HC * BC], F16, tag="acch")
                        acc_c = apool.tile([128, HC * BC], F16, tag="accc")
                        if hot2:
                            for kc in range(HC):
                                sl = slice(kc * BC, (kc + 1) * BC)
                                eng.tensor_add(
                                    acc_h[:, sl],
                                    h_tiles[(terms2[0], l)][:, sl],
                                    h_tiles[(terms2[1], l)][:, sl])
                                for s in terms2[2:]:
                                    eng.tensor_add(
                                        acc_h[:, sl], acc_h[:, sl],
                                        h_tiles[(s, l)][:, sl])
                            for kc in range(HC):
                                sl = slice(kc * BC, (kc + 1) * BC)
                                eng.tensor_add(
                                    acc_c[:, sl],
                                    c_tiles[(terms2[0], l)][:, sl],
                                    c_tiles[(terms2[1], l)][:, sl])
                                for s in terms2[2:]:
                                    eng.tensor_add(
                                        acc_c[:, sl], acc_c[:, sl],
                                        c_tiles[(s, l)][:, sl])
                        else:
                            eng.tensor_add(
                                acc_h[:, :], h_tiles[(terms2[0], l)][:, :],
                                h_tiles[(terms2[1], l)][:, :])
                            eng.tensor_add(
                                acc_c[:, :], c_tiles[(terms2[0], l)][:, :],
                                c_tiles[(terms2[1], l)][:, :])
                            for s in terms2[2:]:
                                eng.tensor_add(
                                    acc_h[:, :], acc_h[:, :],
                                    h_tiles[(s, l)][:, :])
                                eng.tensor_add(
                                    acc_c[:, :], acc_c[:, :],
                                    c_tiles[(s, l)][:, :])
                        h_in = acc_h
                        c_sum = acc_c
                agg_tiles[(i2, l)] = (h_in, c_sum)

            # PE warmup: ~5us of dummy matmuls during the initial weight DMA
            # wait so the HAM clock gate reaches 2.4 GHz before step 0.
            wu_src = kpool.tile([128, BC], F16, tag="wu")
            nc.vector.memset(wu_src[:, :], 0.0)
            for _ in range(3):
                wu_ps = ppool.tile([128, BC], F32, tag="gp0")
                for j in range(8):
                    nc.tensor.matmul(wu_ps[:, :], wu_src[:, 0:128],
                                     wu_src[:, :], start=(j == 0),
                                     stop=(j == 7))

            for k, i in enumerate(live_list):
                if k + 2 < len(live_list):
                    fetch_x(live_list[k + 2])
                nxt = live_list[k + 1] if k + 1 < len(live_list) else None
                w, cnt = sched[i]
                slots = sorted(w.keys())
                # expand multiplicities m_s (integer mask weights)
                terms = []
                for s in slots:
                    terms += [s] * max(int(round(w[s])), 1)
                inv = 1.0 / cnt

                # (the aggregation for this step was emitted at the end of
                # the PREVIOUS step's matching layer body via emit_agg, so
                # its adds queue right behind the producing h2/c2 and run
                # one layer earlier)
                if k == 0:
                    emit_agg(i, 0)
                    emit_agg(i, 1)

                h_l0_new = None
                vkey = int(round(cnt)) if (terms and cnt > 1.0) else 1
                for l in range(L):
                    wih = wihT0 if l == 0 else wihT1
                    whh = whh_v[(l, vkey)]
                    bias = b0 if l == 0 else b1
                    if l == 0:
                        x_chunks = [x_tiles[i][:, :]]
                    else:
                        x_chunks = [h_l0_new[:, kc * BC:(kc + 1) * BC]
                                    for kc in range(HC)]
                    h_in, c_sum = agg_tiles.pop((i, l))

                    # gate matmuls + activations per 128-wide 4H chunk, in
                    # f,f,i,i,g,g,o,o order with the DVE combine interleaved
                    # so it starts while later chunks are still on the PE.
                    gact = gpool.tile([128, NM * BC], F16, tag="gact")

                    ptag = "gp0" if l == 0 else "gp1"

                    def emit_chunk(m):
                        pre = xpre.pop((i, m), None) if l == 0 else None
                        group = []
                        if pre is None:
                            pool = ppool if l == 0 else qpool
                            ps = pool.tile([128, BC], F32, tag=ptag)
                            for kc, xch in enumerate(x_chunks):
                                group.append((wih[:, kc * G4 + m * 128:
                                                  kc * G4 + (m + 1) * 128],
                                              xch, kc == 0))
                        else:
                            ps = pre            # x-part already accumulated
                        if h_in is not None:
                            for kc in range(HC):
                                group.append((whh[:, kc * G4 + m * 128:
                                                  kc * G4 + (m + 1) * 128],
                                              h_in[:, kc * BC:(kc + 1) * BC],
                                              False))
                        for j, (lhsT, rhs, st) in enumerate(group):
                            nc.tensor.matmul(ps[:, :], lhsT, rhs,
                                             start=st,
                                             stop=(j == len(group) - 1),
                                             skip_group_check=True)
                        func = TANH if m in (4, 5) else SIG
                        nc.scalar.activation(gact[:, m * BC:(m + 1) * BC],
                                             ps[:, :], func,
                                             bias=bias[:, m:m + 1])

                    sigi = gact[:, 0 * BC:2 * BC]
                    sigf = gact[:, 2 * BC:4 * BC]
                    tg = gact[:, 4 * BC:6 * BC]
                    sigo = gact[:, 6 * BC:8 * BC]
                    col = color[i + 1]
                    c_new = spool.tile([128, HC * BC], F16, tag=f"c{col}_{l}")

                    # The combine tail runs at H-chunk-half granularity: half
                    # kc only needs o-gate chunk 6+kc and produces the half of
                    # h_new that feeds the next layer's kc-chunk matmuls.
                    th = kpool.tile([128, HC * BC], F16, tag="th")
                    h_new = spool.tile([128, HC * BC], F16, tag=f"h{col}_{l}")
                    B2 = BC                        # 512 cols per half

                    def half(ap, kc):
                        return ap[:, kc * B2:(kc + 1) * B2]

                    if c_sum is None:
                        # no predecessors: c_in = 0, so sigf is irrelevant --
                        # skip the f-gate chunks (2,3) entirely.
                        for m in (0, 1, 4, 5):
                            emit_chunk(m)
                        for kc in range(HC):
                            nc.vector.tensor_mul(half(c_new, kc),
                                                 half(sigi, kc), half(tg, kc))
                            nc.scalar.activation(half(th, kc),
                                                 half(c_new, kc), TANH)
                            emit_chunk(6 + kc)
                            nc.vector.tensor_mul(half(h_new, kc),
                                                 half(sigo, kc), half(th, kc))
                    else:
                        # f gate FIRST so the c path starts while the rest of
                        # the gate chunks are still streaming on the PE.
                        for m in (2, 3):
                            emit_chunk(m)
                        for kc in range(HC):
                            sl = slice(kc * BC, (kc + 1) * BC)
                            if cnt == 1.0:
                                nc.vector.tensor_mul(c_new[:, sl],
                                                     sigf[:, sl],
                                                     c_sum[:, sl])
                            else:
                                # c_new = (c_sum * 1/cnt) * sigf, fused
                                nc.vector.scalar_tensor_tensor(
                                    c_new[:, sl], c_sum[:, sl], inv,
                                    sigf[:, sl], ALU.mult, ALU.mult)
                        for m in (0, 1, 4, 5):     # i and g gates
                            emit_chunk(m)
                        t2 = kpool.tile([128, HC * BC], F16, tag="t2")
                        for kc in range(HC):
                            nc.vector.tensor_mul(half(t2, kc),
                                                 half(sigi, kc), half(tg, kc))
                            nc.vector.tensor_add(half(c_new, kc),
                                                 half(c_new, kc),
                                                 half(t2, kc))
                            nc.scalar.activation(half(th, kc),
                                                 half(c_new, kc), TANH)
                            emit_chunk(6 + kc)
                            nc.vector.tensor_mul(half(h_new, kc),
                                                 half(sigo, kc), half(th, kc))

                    h_tiles[(i + 1, l)] = h_new
                    c_tiles[(i + 1, l)] = c_new
                    if nxt is not None:
                        emit_agg(nxt, l)
                    if l == 0:
                        h_l0_new = h_new
                        # Software-pipelined x-projection for step i+1 layer
                        # 0: depends only on the DMA'd x tile, so the PE can
                        # run it during this step's combine tails. The psum
                        # groups stay open; step i+1's h-part matmuls join
                        # them (start=False) and close the group.
                        if nxt is not None:
                            w1 = sched[nxt][0]
                            t1list = []
                            for s1 in sorted(w1):
                                t1list += [s1] * max(int(round(w1[s1])), 1)
                            pset = (2, 3, 0, 1) if t1list else (0, 1, 4, 5)
                            for m in pset:
                                ps = ppool.tile([128, BC], F32, tag="gp0")
                                nc.tensor.matmul(
                                    ps[:, :],
                                    wihT0[:, m * 128:(m + 1) * 128],
                                    x_tiles[nxt][:, :], start=True,
                                    stop=(not t1list),
                                    skip_group_check=True)
                                xpre[(nxt, m)] = ps

            # ---- MLP ------------------------------------------------------
            hlast = h_tiles[(N, L - 1)]
            fc_chunks = [hlast[:, 0:BC], hlast[:, BC:2 * BC], featT[:, :]]

            a0 = gpool.tile([128, 4 * BC], F16, tag="gact")
            for mo in range(4):
                ps = ppool.tile([128, BC], F32, tag="gp0")
                for j, fch in enumerate(fc_chunks):
                    nc.tensor.matmul(
                        ps[:, :],
                        mw0T[:, j * MH + mo * 128: j * MH + (mo + 1) * 128],
                        fch, start=(j == 0), stop=(j == len(fc_chunks) - 1))
                nc.scalar.activation(a0[:, mo * BC:(mo + 1) * BC], ps[:, :],
                                     AF.Relu, bias=mb0[:, mo:mo + 1])

            a1 = gpool.tile([128, 4 * BC], F16, tag="gact")
            for mo in range(4):
                ps = qpool.tile([128, BC], F32, tag="gp1")
                for kc in range(4):
                    nc.tensor.matmul(
                        ps[:, :],
                        mw1T[:, kc * MH + mo * 128: kc * MH + (mo + 1) * 128],
                        a0[:, kc * BC:(kc + 1) * BC],
                        start=(kc == 0), stop=(kc == 3))
                nc.scalar.activation(a1[:, mo * BC:(mo + 1) * BC], ps[:, :],
                                     AF.Relu, bias=mb1[:, mo:mo + 1])

            psf = ppool.tile([128, BC], F32, tag="gp0")
            for kc in range(4):
                nc.tensor.matmul(psf[:1, :], mw2T[:, kc:kc + 1],
                                 a1[:, kc * BC:(kc + 1) * BC],
                                 start=(kc == 0), stop=(kc == 3))
            out_sb = kpool.tile([128, BC], F32, tag="th")
            nc.scalar.activation(out_sb[:1, :], psf[:1, :], AF.Identity,
                                 bias=mb2[:1, 0:1])
            nc.sync.dma_start(out=d_out[:, :], in_=out_sb[:1, :])

    nc.compile()
    return nc


def _prep_core_inputs(inputs):
    """Host-side layout prep shared by all cores + per-core slices."""
    f16 = np.float16
    f32 = np.float32

    wihT0 = _chunk_k(np.ascontiguousarray(inputs["Wih0"].T)).astype(f16)
    whhT0f = _chunk_k(np.ascontiguousarray(inputs["Whh0"].T))
    whhT0 = whhT0f.astype(f16)
    wihT1 = _chunk_k(np.ascontiguousarray(inputs["Wih1"].T)).astype(f16)
    whhT1f = _chunk_k(np.ascontiguousarray(inputs["Whh1"].T))
    whhT1 = whhT1f.astype(f16)

    sched = _dag_schedule(np.asarray(inputs["pred_idx"], np.int32),
                          np.asarray(inputs["pred_mask"], np.int32))
    var_arrays = {}
    for v in _needed_variants(sched):
        var_arrays[f"whhT0_v{v}"] = (whhT0f / v).astype(f16)
        var_arrays[f"whhT1_v{v}"] = (whhT1f / v).astype(f16)
    b0 = np.ascontiguousarray((inputs["bih0"] + inputs["bhh0"])
                              .astype(f32).reshape(NM, 128).T)
    b1 = np.ascontiguousarray((inputs["bih1"] + inputs["bhh1"])
                              .astype(f32).reshape(NM, 128).T)
    mw0T = _chunk_k(np.ascontiguousarray(inputs["mW0"].T)).astype(f16)
    mw1T = _chunk_k(np.ascontiguousarray(inputs["mW1"].T)).astype(f16)
    mw2T = _chunk_k(np.ascontiguousarray(inputs["mW2"].T)).astype(f16)
    mb0 = np.ascontiguousarray(inputs["mb0"].astype(f32).reshape(4, 128).T)
    mb1 = np.ascontiguousarray(inputs["mb1"].astype(f32).reshape(4, 128).T)
    mb2 = np.zeros((128, 1), f32)
    mb2[0, 0] = np.float32(inputs["mb2"][0])

    shared = dict(wihT0=wihT0, whhT0=whhT0, wihT1=wihT1, whhT1=whhT1,
                  b0=b0, b1=b1, mw0T=mw0T, mw1T=mw1T, mw2T=mw2T,
                  mb0=mb0, mb1=mb1, mb2=mb2, **var_arrays)

    dags = np.asarray(inputs["dags"], np.float32)
    feats = np.asarray(inputs["features"], np.float32)
    in_maps = []
    for c in range(NCORES):
        lo, hi = c * BC, (c + 1) * BC
        dagsT = np.ascontiguousarray(
            dags[lo:hi].transpose(1, 2, 0)).astype(f16)      # [N, IN, BC]
        featT = np.ascontiguousarray(feats[lo:hi].T).astype(f16)
        m = dict(shared)
        m["dagsT"] = dagsT
        m["featT"] = featT
        in_maps.append(m)
    return in_maps


def _get_nc(pred_idx, pred_mask):
    key = (pred_idx.tobytes(), pred_mask.tobytes())
    if key not in _BUILD_CACHE:
        _BUILD_CACHE[key] = _build(pred_idx, pred_mask)
    return _BUILD_CACHE[key]


def run(inputs, trace=False):
    from concourse.bass_utils import run_bass_kernel_spmd

    pred_idx = np.asarray(inputs["pred_idx"], np.int32)
    pred_mask = np.asarray(inputs["pred_mask"], np.int32)
    nc = _get_nc(pred_idx, pred_mask)
    in_maps = _prep_core_inputs(inputs)
    res = run_bass_kernel_spmd(nc, in_maps, core_ids=list(range(NCORES)),
                               trace=trace)
    out = np.concatenate([np.asarray(r["out"], np.float32).reshape(BC)
                          for r in res.results])
    return out, res


def kernel(**inputs) -> np.ndarray:
    out, _ = run(inputs, trace=False)
    return out
